# revision 39
# baseline (speedup 1.0000x reference)
"""Trainium2 Bass kernel for nn_Attention_46780783788294.

Multi-head causal-ish attention (mask fills with 0.0, not -inf) for
x:[2,2048,1024], 16 heads of d_head=64, fp32 in/out, bf16 compute.

Sharding: 8 cores = 2 batches x 4 head-groups (4 heads each). Each core
computes its batch/head-group partial output [2048,1024] (fp32); host sums
the 4 partials per batch and adds b_O.

Per-core device program (all-transposed "S^T" layout, bf16 matmuls with
fp32 PSUM accumulation):
  xT_aug [1025,2048] (x^T plus ones row) and packed/augmented weights come
  from the host. QT/KT computed per head-pair [128,2048] (d on partitions);
  V computed in natural [k,d] layout [128,260] per k-block with a per-head
  ones column (from the bias-row trick) so the AV matmul accumulates the
  softmax denominator for free. Scores are built k-on-partitions so
  P = exp(mask * s / 8) feeds the AV matmul directly with no transposes;
  masked entries give exp(0)=1 exactly as the reference's 0.0-fill softmax
  requires. Fully-masked regions of P (the column prefix of diagonal
  k-blocks) are not computed at all: P=1 there, so the prefix is memset to
  1.0 on the (otherwise idle) GPSIMD engine and the AV matmul runs full
  width — no analytic-correction matmuls needed. The two heads of a pair
  share one [128,1024] scores psum so each mask-mul/exp covers both heads
  in one instruction; the two 64-contraction score matmuls row-pack into
  disjoint halves of the PE array and run concurrently.

  Softmax denominators ride in the AV psum at partition 64; they are
  reciprocal'd straight out of PSUM into partitions 64/96 of a staging
  tile, broadcast across partitions with a single 33-row matmul, and the
  divide is fused into the z psum->sbuf extraction copy. The output
  projection result is DMA'd to DRAM directly from PSUM in fp32.

  Chunk-outer loop: each q-chunk is attended, divided, output-projected
  and DMA'd out before the next, so PE/ACT/DVE/DMA stay overlapped. A
  burst of dummy warm-up matmuls at t=0 keeps the PE clock un-throttled
  (HAM) through the initial weight-DMA wait.
"""

import os
import sys

import numpy as np


def _ensure_concourse():
    try:
        import concourse  # noqa: F401
    except ImportError:
        for p in ("/root/.axon_site", "/root/.axon_site/_ro/trn_rl_repo",
                  "/root/.axon_site/_ro/pypackages", "/opt/trn_rl_repo"):
            if os.path.isdir(p) and p not in sys.path:
                sys.path.append(p)


_ensure_concourse()

import concourse.bass as bass  # noqa: E402
import concourse.tile as tile  # noqa: E402
from concourse import bacc, mybir  # noqa: E402
from concourse import bass_utils  # noqa: E402
from contextlib import ExitStack  # noqa: E402

F32 = mybir.dt.float32
BF16 = mybir.dt.bfloat16
EXP = mybir.ActivationFunctionType.Exp

S = 2048      # sequence length
M = 1024      # d_model
DH = 64       # d_head
HL = 4        # heads per core
NP = 2        # head pairs per core
CH = 512      # q-chunk width
NCH = S // CH     # 4 q chunks
KB = S // 128     # 16 k blocks
MB = M // 128     # 8 m blocks
N_CORES = 8
WARMUP = 34   # dummy N=128 matmuls at t=0 to keep HAM warm through DMA wait


def _emit(tc, nc, d, zero_bias):
    mm = nc.tensor.matmul
    with ExitStack() as ctx:
        # ---- persistent pools ----
        qkp = ctx.enter_context(tc.tile_pool(name="qkp", bufs=1))
        vp = ctx.enter_context(tc.tile_pool(name="vp", bufs=1))
        wop = ctx.enter_context(tc.tile_pool(name="wop", bufs=1))
        cst = ctx.enter_context(tc.tile_pool(name="cst", bufs=1))
        z2p = ctx.enter_context(tc.tile_pool(name="z2p", bufs=1))
        xp = ctx.enter_context(tc.tile_pool(name="xp", bufs=1))
        wp = ctx.enter_context(tc.tile_pool(name="wp", bufs=1))
        pp = ctx.enter_context(tc.tile_pool(name="pp", bufs=6))
        op_sb = ctx.enter_context(tc.tile_pool(name="op_sb", bufs=4))
        stg = ctx.enter_context(tc.tile_pool(name="stg", bufs=2))
        # PSUM budget is 8 banks, statically split: psS 2x2, psZ 2x1, and a
        # single 2-slot pool shared by every 1-bank psum use
        psX = ctx.enter_context(tc.tile_pool(name="psX", bufs=2, space="PSUM"))
        psS = ctx.enter_context(tc.tile_pool(name="psS", bufs=2, space="PSUM"))
        psZ = ctx.enter_context(tc.tile_pool(name="psZ", bufs=1, space="PSUM"))

        qt = [qkp.tile([128, S], BF16, name=f"qt{p}") for p in range(NP)]
        kt = [qkp.tile([128, S], BF16, name=f"kt{p}") for p in range(NP)]
        vt = [vp.tile([128, 260], BF16, name=f"vt{j}") for j in range(KB)]
        wo_t = [wop.tile([128, M], BF16, name=f"wo{p}") for p in range(NP)]
        dtri = cst.tile([128, 256], BF16, name="dtri")
        e33 = cst.tile([97, 128], BF16, name="e33")
        wu = cst.tile([128, 128], BF16, name="wu")
        # per-pair reciprocal denominators, bf16, at partitions 64 (half 0)
        # and 96 (half 1); rows 65..95 stay zero so the e33 broadcast matmul
        # sees finite values under its zero weights
        rdcb = cst.tile([97, NP * CH], BF16, name="rdcb")
        # reciprocal staging lives entirely at partition 0: the custom DVE
        # reciprocal op only behaves correctly with zero partition offset.
        # Column layout: [p0h0 | p0h1 | p1h0 | p1h1], 512 each.
        rdcf_a = cst.tile([1, NP * 2 * CH], F32, name="rdcf_a")
        rdcf_b = cst.tile([1, NP * 2 * CH], F32, name="rdcf_b")
        # per-(pair,half) suffix V column-sums: partition d (base 0), col =
        # chunk index. Computed on device by projecting host-side suffix
        # sums of x through W_V (plus counts*b_V via the ones-row trick).
        # Kept per-half at base partition 0: scalar_tensor_tensor requires
        # equal base partitions for its SBUF inputs.
        sfxT = [[cst.tile([64, 4], F32, name=f"sfxT{p}_{h}") for h in range(2)]
                for p in range(NP)]
        xsum_t = xp.tile([128, 4 * MB], BF16, name="xsum")
        if not zero_bias:
            xsum_ones = xp.tile([1, 4], BF16, name="xsum_ones")
        z2u = [z2p.tile([128, S], BF16, name=f"z2u{p}") for p in range(NP)]

        xt_t = [xp.tile([128, S], BF16, name=f"xt{mb}") for mb in range(MB)]
        wq_t = [wp.tile([128, 256], BF16, name=f"wq{mb}") for mb in range(MB)]
        wk_t = [wp.tile([128, 256], BF16, name=f"wk{mb}") for mb in range(MB)]
        wv_t = [wp.tile([128, 256], BF16, name=f"wv{mb}") for mb in range(MB)]
        if not zero_bias:
            xt_ones = xp.tile([1, S], BF16, name="xt_ones")
            wq_b = wp.tile([1, 256], BF16, name="wq_b")
            wk_b = wp.tile([1, 256], BF16, name="wk_b")
            wv_b = wp.tile([1, 256], BF16, name="wv_b")

        # ---- PE warm-up: dummy matmuls on a zeroed tile keep the PE busy
        # (and the HAM clock-gate open) while the first weight DMAs land ----
        nc.vector.memset(wu[:], 0.0)
        nc.vector.memset(e33[64:97, :], 0.0)
        nc.vector.memset(e33[64:65, 0:64], 1.0)
        nc.vector.memset(e33[96:97, 64:128], 1.0)
        nc.vector.memset(rdcb[64:97, :], 0.0)
        for w in range(WARMUP):
            wps = psX.tile([128, 128], F32, name="wps", tag="px")
            mm(wps[:], wu[:], wu[:], start=True, stop=True)

        # DMA order: what attention chunk 0 needs first (wq/wk, x^T chunk 0,
        # wv, masks), then the rest of x^T; W_O last. Intro loads are split
        # across both HWDGE dispatch engines (sync + scalar) since the
        # scalar engine is idle until the first exp.
        for mb in range(MB):
            ea = nc.scalar if mb % 2 else nc.sync
            eb = nc.sync if mb % 2 else nc.scalar
            ea.dma_start(wq_t[mb][:], d["wq"][128 * mb:128 * (mb + 1), :])
            if mb < 2:
                # first tiles gate the first matmuls: halve their latency by
                # loading each on two queues
                eb.dma_start(xt_t[mb][:, 0:256],
                             d["xt"][128 * mb:128 * (mb + 1), 0:256])
                ea.dma_start(xt_t[mb][:, 256:CH],
                             d["xt"][128 * mb:128 * (mb + 1), 256:CH])
            else:
                eb.dma_start(xt_t[mb][:, 0:CH],
                             d["xt"][128 * mb:128 * (mb + 1), 0:CH])
            ea.dma_start(wk_t[mb][:], d["wk"][128 * mb:128 * (mb + 1), :])
        for mb in range(MB):
            eng = nc.sync if mb % 2 else nc.scalar
            eng.dma_start(wv_t[mb][:], d["wv"][128 * mb:128 * (mb + 1), :])
        if not zero_bias:
            nc.sync.dma_start(wq_b[:], d["wq"][1024:1025, :])
            nc.sync.dma_start(wk_b[:], d["wk"][1024:1025, :])
            nc.sync.dma_start(wv_b[:], d["wv"][1024:1025, :])
            nc.sync.dma_start(xt_ones[:], d["xt"][1024:1025, :])
        nc.scalar.dma_start(dtri[:], d["mk"][:])
        nc.scalar.dma_start(
            xsum_t.rearrange("p (m c) -> p m c", c=4),
            d["xs"][0:1024, :].rearrange("(m p) c -> p m c", p=128))
        if not zero_bias:
            nc.scalar.dma_start(xsum_ones[:], d["xs"][1024:1025, :])
        for c in range(1, NCH):
            for mb in range(MB):
                nc.sync.dma_start(
                    xt_t[mb][:, CH * c:CH * (c + 1)],
                    d["xt"][128 * mb:128 * (mb + 1), CH * c:CH * (c + 1)])
        for p in range(NP):
            nc.sync.dma_start(wo_t[p][:], d["wo"][128 * p:128 * (p + 1), :])

        def emit_v(j0):
            # two k-blocks share one [128,512] psum bank: halves the shared
            # psum slot churn. V is packed head-major (256 cols); the
            # per-head ones columns of vt are memset separately.
            ps = psX.tile([128, 2 * 256], F32, name="psv", tag="px")
            for dj in range(2):
                j = j0 + dj
                for mb in range(MB):
                    mm(ps[:, 256 * dj:256 * (dj + 1)],
                       xt_t[mb][:, 128 * j:128 * (j + 1)],
                       wv_t[mb][:], start=(mb == 0),
                       stop=(zero_bias and mb == MB - 1))
                if not zero_bias:
                    mm(ps[:, 256 * dj:256 * (dj + 1)],
                       xt_ones[:, 128 * j:128 * (j + 1)], wv_b[:],
                       start=False, stop=True)
            for dj in range(2):
                j = j0 + dj
                pssl = ps[:, 256 * dj:256 * (dj + 1)]
                nc.vector.tensor_copy(
                    vt[j].rearrange("p (h c) -> p h c", c=65)[:, :, 0:64],
                    pssl.rearrange("p (h c) -> p h c", c=64))
                oc = vt[j].rearrange("p (h c) -> p h c", c=65)[:, :, 64]
                nc.gpsimd.memset(oc, 1.0)

        def emit_qk(p, which, c):
            dst, wt = (qt, wq_t) if which == 0 else (kt, wk_t)
            ps = psX.tile([128, CH], F32, name="psqk", tag="px")
            for mb in range(MB):
                mm(ps[:], wt[mb][:, 128 * p:128 * (p + 1)],
                   xt_t[mb][:, CH * c:CH * (c + 1)],
                   start=(mb == 0), stop=(zero_bias and mb == MB - 1))
            if not zero_bias:
                wb = wq_b if which == 0 else wk_b
                mm(ps[:], wb[:, 128 * p:128 * (p + 1)],
                   xt_ones[:, CH * c:CH * (c + 1)], start=False, stop=True)
            nc.vector.tensor_copy(dst[p][:, CH * c:CH * (c + 1)], ps[:])

        zps_by_pair = {}

        def emit_attn_pair(ch, p):
            # scores/mask/exp/AV for all k-blocks of chunk ch, head pair p
            nj = 4 * ch + 4
            if True:
                h0, h1 = 2 * p, 2 * p + 1
                zps = [psZ.tile([65, CH], F32, name=f"zps{half}",
                                tag=f"zps{half}") for half in range(2)]
                zps_by_pair[(ch, p)] = zps
                for j in range(nj):
                    # both heads of the pair share one [128,1024] scores
                    # psum: one mask-mul + one exp per j. For diagonal
                    # blocks only the unmasked column suffix [w0:512) is
                    # computed; the fully-masked prefix of P is exp(0)=1
                    # exactly, so it is memset to 1.0 (GPSIMD) instead.
                    r = j - 4 * ch
                    w0 = 128 * r if r > 0 else 0
                    last = j == nj - 1
                    sps = psS.tile([128, 2 * CH], F32, name="sps", tag="sps")
                    mm(sps[:, w0:CH],
                       kt[p][0:64, 128 * j:128 * (j + 1)],
                       qt[p][0:64, CH * ch + w0:CH * (ch + 1)],
                       start=True, stop=True)
                    mm(sps[:, CH + w0:2 * CH],
                       kt[p][64:128, 128 * j:128 * (j + 1)],
                       qt[p][64:128, CH * ch + w0:CH * (ch + 1)],
                       start=True, stop=True)
                    sps3 = sps.rearrange("p (t c) -> p t c", t=2)
                    pt = pp.tile([128, 2 * CH], BF16, name="pt", tag="pt")
                    pt3 = pt.rearrange("p (t c) -> p t c", t=2)
                    if r >= 0:
                        # triangular mask on the 128-wide diagonal strip of
                        # both heads at once
                        strip = sps3[:, :, w0:w0 + 128]
                        dtri3 = dtri.rearrange("p (t c) -> p t c", t=2)
                        nc.vector.tensor_mul(strip, strip, dtri3)
                    if w0:
                        nc.gpsimd.memset(pt3[:, :, 0:w0], 1.0)
                        nc.scalar.activation(pt3[:, :, w0:CH],
                                             sps3[:, :, w0:CH], EXP,
                                             scale=0.125)
                    elif j == 0:
                        # at pair/chunk starts, exp the two heads separately
                        # so the first AV matmul starts half an exp earlier
                        nc.scalar.activation(pt[:, 0:CH], sps[:, 0:CH],
                                             EXP, scale=0.125)
                        nc.scalar.activation(pt[:, CH:2 * CH],
                                             sps[:, CH:2 * CH],
                                             EXP, scale=0.125)
                    else:
                        nc.scalar.activation(pt[:], sps[:], EXP, scale=0.125)
                    mm(zps[0][:], vt[j][:, 65 * h0:65 * h0 + 65],
                       pt[:, 0:CH], start=(j == 0), stop=last)
                    mm(zps[1][:], vt[j][:, 65 * h1:65 * h1 + 65],
                       pt[:, CH:2 * CH], start=(j == 0), stop=last)

        def emit_sfx():
            # project host-side suffix sums of x through W_V: per pair a
            # [128,4] psum with partition hb+d, col = chunk index, holding
            # sum_{k in suffix(ch)} V[k, d] (+ count*b_V via the ones row)
            for p in range(NP):
                ps = psX.tile([128, 4], F32, name="pssfx", tag="px")
                for mb in range(MB):
                    mm(ps[:], wv_t[mb][:, 128 * p:128 * (p + 1)],
                       xsum_t[:, 4 * mb:4 * (mb + 1)],
                       start=(mb == 0), stop=(zero_bias and mb == MB - 1))
                if not zero_bias:
                    mm(ps[:], wv_b[:, 128 * p:128 * (p + 1)],
                       xsum_ones[:], start=False, stop=True)
                for half in range(2):
                    nc.vector.tensor_copy(sfxT[p][half][:],
                                          ps[64 * half:64 * half + 64, :])

        stg_by_pair = {}

        def emit_zstage(ch, p, tail=False):
            # release-critical: one [65,512] PSUM->SBUF copy per half frees
            # the z psum banks; everything else (reciprocal, broadcast,
            # divide) reads the SBUF staging copy later, off this path.
            # At the tail the scalar engine is idle (no more exps), so the
            # copies run there instead of the loaded vector engine.
            zl = zps_by_pair.pop((ch, p))
            sg = [stg.tile([65, CH], F32, name=f"sg{half}", tag=f"sg{half}")
                  for half in range(2)]
            for half in range(2):
                if tail:
                    nc.scalar.activation(sg[half][:], zl[half][:],
                                         mybir.ActivationFunctionType.Copy)
                else:
                    nc.vector.tensor_copy(sg[half][:], zl[half][:])
            stg_by_pair[(ch, p)] = sg

        def emit_zdiv(ch, p, tail=False):
            # deferred division for (chunk, pair): denominator (+ suffix
            # count) -> reciprocal at partition 0 -> bf16 rows 64/96 ->
            # one 33-row broadcast matmul -> fused (z + sfx) * (1/d) into
            # z2u. Fully-masked suffix k-blocks contribute P=1 each: a
            # count to the denominator and sfxT to z.
            sg = stg_by_pair.pop((ch, p))
            cnt = float(S - CH * (ch + 1))
            rsl = rdcb[:, CH * p:CH * (p + 1)]
            ra = rdcf_a[:, 2 * CH * p:2 * CH * (p + 1)]
            rb = rdcf_b[:, 2 * CH * p:2 * CH * (p + 1)]
            for half in range(2):
                if tail:
                    nc.scalar.activation(ra[0:1, CH * half:CH * (half + 1)],
                                         sg[half][64:65, :],
                                         mybir.ActivationFunctionType.Copy,
                                         bias=cnt)
                else:
                    nc.vector.tensor_scalar_add(
                        ra[0:1, CH * half:CH * (half + 1)],
                        sg[half][64:65, :], cnt)
            nc.vector.reciprocal_approx_fast(rb[0:1, :], ra[0:1, :])
            for half, row in ((0, 64), (1, 96)):
                nc.vector.tensor_copy(rsl[row:row + 1, :],
                                      rb[0:1, CH * half:CH * (half + 1)])
            bc = psX.tile([128, CH], F32, name="bc", tag="px")
            mm(bc[:], e33[64:97, :], rsl[64:97, :], start=True, stop=True)
            for half in range(2):
                hb = 64 * half
                dst = z2u[p][hb:hb + 64, CH * ch:CH * (ch + 1)]
                if cnt:
                    nc.vector.scalar_tensor_tensor(
                        dst, sg[half][0:64, :],
                        sfxT[p][half][:, ch:ch + 1], bc[hb:hb + 64, :],
                        mybir.AluOpType.add, mybir.AluOpType.mult)
                else:
                    nc.vector.tensor_mul(dst, sg[half][0:64, :],
                                         bc[hb:hb + 64, :])

        def emit_divE(ch, tail=False):
            # project chunk ch's z to the output and stream to DRAM. Out
            # DMAs ride the sync queue (dispatching them on the scalar
            # queue delays exp, which gates attention); at the tail the
            # scalar engine+queue are idle, so half the casts and DMAs
            # move there to halve the drain.
            for q in range(4 * ch, 4 * ch + 4):
                for mc in range(2):
                    ops = psX.tile([128, CH], F32, name="ops", tag="px")
                    for p in range(NP):
                        mm(ops[:], z2u[p][:, 128 * q:128 * (q + 1)],
                           wo_t[p][:, CH * mc:CH * (mc + 1)],
                           start=(p == 0), stop=(p == 1))
                    osb = op_sb.tile([128, CH], BF16, name="osb", tag="osb")
                    if tail and mc:
                        nc.scalar.activation(
                            osb[:], ops[:], mybir.ActivationFunctionType.Copy)
                    else:
                        nc.vector.tensor_copy(osb[:], ops[:])
                    eng = nc.scalar if (tail and mc) else nc.sync
                    eng.dma_start(
                        d["out"][128 * q:128 * (q + 1), CH * mc:CH * (mc + 1)],
                        osb[:])

        # ---- emission: emission order doubles as scheduler priority.
        # Per chunk: attention pair 0, its release copies, then the
        # previous chunk's deferred division for pair 0 (fills DVE during
        # this chunk's attention); same for pair 1; then next chunk's Q/K
        # (PE filler at the boundary) and the previous chunk's output
        # projection. The division chains and O-proj run a full pair/chunk
        # behind the attention that produced their data, so the z psum
        # handoff between pairs only ever waits on the two staging copies.
        for p in range(NP):
            emit_qk(p, 0, 0)
            emit_qk(p, 1, 0)
        for j in range(0, 4, 2):
            emit_v(j)
        emit_attn_pair(0, 0)
        for j in range(4, KB, 2):
            emit_v(j)
        emit_sfx()
        emit_zstage(0, 0)
        emit_attn_pair(0, 1)
        emit_zstage(0, 1)
        for ch in range(1, NCH):
            for p in range(NP):
                emit_qk(p, 0, ch)
                emit_qk(p, 1, ch)
            emit_attn_pair(ch, 0)
            emit_zstage(ch, 0)
            emit_zdiv(ch - 1, 0)
            emit_attn_pair(ch, 1)
            emit_zstage(ch, 1, tail=(ch == 3))
            emit_zdiv(ch - 1, 1)
            emit_divE(ch - 1)
        emit_zdiv(3, 0)
        emit_zdiv(3, 1, tail=True)
        emit_divE(3, tail=True)


def build_program(zero_bias=False):
    nc = bacc.Bacc("TRN2", target_bir_lowering=False, debug=False,
                   num_devices=N_CORES)
    d = {
        "xt": nc.dram_tensor("xt", [1025, S], BF16, kind="ExternalInput").ap(),
        "wq": nc.dram_tensor("wq", [1025, 256], BF16, kind="ExternalInput").ap(),
        "wk": nc.dram_tensor("wk", [1025, 256], BF16, kind="ExternalInput").ap(),
        "wv": nc.dram_tensor("wv", [1025, 256], BF16, kind="ExternalInput").ap(),
        "wo": nc.dram_tensor("wo", [256, M], BF16, kind="ExternalInput").ap(),
        "mk": nc.dram_tensor("mk", [128, 256], BF16, kind="ExternalInput").ap(),
        "xs": nc.dram_tensor("xs", [1025, 4], BF16, kind="ExternalInput").ap(),
        "out": nc.dram_tensor("out", [S, M], BF16, kind="ExternalOutput").ap(),
    }
    with tile.TileContext(nc) as tc:
        _emit(tc, nc, d, zero_bias)
    nc.compile()
    return nc


_CACHE = {}


def _get_program(zero_bias=False):
    key = ("nc", zero_bias)
    if key not in _CACHE:
        _CACHE[key] = build_program(zero_bias)
    return _CACHE[key]


def _pack_qk(w4, b4):
    # w4 [4,1024,64], b4 [4,64] -> [1025, 256] (m-major, head-major cols)
    r = np.empty((1025, 256), np.float32)
    r[:1024] = w4.transpose(1, 0, 2).reshape(1024, 256)
    r[1024] = b4.reshape(256)
    return r


def _pack_v(w4, b4):
    # [1025, 256] head-major; vt ones columns are memset on device
    r = np.empty((1025, 256), np.float32)
    r[:1024] = w4.transpose(1, 0, 2).reshape(1024, 256)
    r[1024] = b4.reshape(256)
    return r


def prepare_in_maps(normalized_resid_pre, W_Q, b_Q, W_K, b_K, W_V, b_V, W_O,
                    b_O):
    import ml_dtypes
    bf16 = ml_dtypes.bfloat16
    x = np.asarray(normalized_resid_pre, np.float32)
    W_Q = np.asarray(W_Q, np.float32)
    b_Q = np.asarray(b_Q, np.float32)
    W_K = np.asarray(W_K, np.float32)
    b_K = np.asarray(b_K, np.float32)
    W_V = np.asarray(W_V, np.float32)
    b_V = np.asarray(b_V, np.float32)
    W_O = np.asarray(W_O, np.float32)

    tri = np.triu(np.ones((128, 128), np.float32))  # [k,q]: 1 where k <= q
    mk = np.tile(tri, (1, 2))  # both heads of a pair side by side

    xts = []
    xss = []
    for b in range(2):
        xt = np.empty((1025, S), np.float32)
        xt[:1024] = x[b].T
        xt[1024] = 1.0
        xts.append(xt.astype(bf16))
        # suffix sums of x over k >= 512*(c+1) (input preprocessing for the
        # on-device fully-masked-suffix correction), plus suffix counts in
        # the ones row so the bias matmul picks up count*b_V
        xs = np.zeros((1025, 4), np.float32)
        for c in range(3):
            xs[:1024, c] = x[b][512 * (c + 1):].sum(axis=0)
            xs[1024, c] = S - 512 * (c + 1)
        xss.append(xs.astype(bf16))

    in_maps = []
    for c in range(N_CORES):
        b, g = divmod(c, 4)
        hs = slice(4 * g, 4 * g + 4)
        in_maps.append({
            "xt": xts[b],
            "wq": _pack_qk(W_Q[hs], b_Q[hs]).astype(bf16),
            "wk": _pack_qk(W_K[hs], b_K[hs]).astype(bf16),
            "wv": _pack_v(W_V[hs], b_V[hs]).astype(bf16),
            "wo": np.ascontiguousarray(W_O[hs].reshape(256, M)).astype(bf16),
            "mk": mk.astype(bf16),
            "xs": xss[b],
        })
    return in_maps


def gather(results, b_O):
    out = np.zeros((2, S, M), np.float32)
    for c in range(N_CORES):
        out[c // 4] += np.asarray(results[c]["out"], dtype=np.float32)
    out += np.asarray(b_O, np.float32)[None, None, :]
    return out


def _run(in_maps, trace=False, zero_bias=False, **kw):
    nc = _get_program(zero_bias)
    return bass_utils.run_bass_kernel_spmd(
        nc, in_maps, core_ids=list(range(N_CORES)), trace=trace, **kw)


def all_zero_bias(b_Q, b_K, b_V):
    return (not np.any(np.asarray(b_Q)) and not np.any(np.asarray(b_K))
            and not np.any(np.asarray(b_V)))


def kernel(normalized_resid_pre, W_Q, b_Q, W_K, b_K, W_V, b_V, W_O, b_O):
    in_maps = prepare_in_maps(normalized_resid_pre, W_Q, b_Q, W_K, b_K, W_V,
                              b_V, W_O, b_O)
    res = _run(in_maps, zero_bias=all_zero_bias(b_Q, b_K, b_V))
    return gather(res.results, b_O)


# revision 40
# speedup vs baseline: 1.0073x; 1.0073x over previous
"""Trainium2 Bass kernel for nn_Attention_46780783788294.

Multi-head causal-ish attention (mask fills with 0.0, not -inf) for
x:[2,2048,1024], 16 heads of d_head=64, fp32 in/out, bf16 compute.

Sharding: 8 cores = 2 batches x 4 head-groups (4 heads each). Each core
computes its batch/head-group partial output [2048,1024] (fp32); host sums
the 4 partials per batch and adds b_O.

Per-core device program (all-transposed "S^T" layout, bf16 matmuls with
fp32 PSUM accumulation):
  xT_aug [1025,2048] (x^T plus ones row) and packed/augmented weights come
  from the host. QT/KT computed per head-pair [128,2048] (d on partitions);
  V computed in natural [k,d] layout [128,260] per k-block with a per-head
  ones column (from the bias-row trick) so the AV matmul accumulates the
  softmax denominator for free. Scores are built k-on-partitions so
  P = exp(mask * s / 8) feeds the AV matmul directly with no transposes;
  masked entries give exp(0)=1 exactly as the reference's 0.0-fill softmax
  requires. Fully-masked regions of P (the column prefix of diagonal
  k-blocks) are not computed at all: P=1 there, so the prefix is memset to
  1.0 on the (otherwise idle) GPSIMD engine and the AV matmul runs full
  width — no analytic-correction matmuls needed. The two heads of a pair
  share one [128,1024] scores psum so each mask-mul/exp covers both heads
  in one instruction; the two 64-contraction score matmuls row-pack into
  disjoint halves of the PE array and run concurrently.

  Softmax denominators ride in the AV psum at partition 64; they are
  reciprocal'd straight out of PSUM into partitions 64/96 of a staging
  tile, broadcast across partitions with a single 33-row matmul, and the
  divide is fused into the z psum->sbuf extraction copy. The output
  projection result is DMA'd to DRAM directly from PSUM in fp32.

  Chunk-outer loop: each q-chunk is attended, divided, output-projected
  and DMA'd out before the next, so PE/ACT/DVE/DMA stay overlapped. A
  burst of dummy warm-up matmuls at t=0 keeps the PE clock un-throttled
  (HAM) through the initial weight-DMA wait.
"""

import os
import sys

import numpy as np


def _ensure_concourse():
    try:
        import concourse  # noqa: F401
    except ImportError:
        for p in ("/root/.axon_site", "/root/.axon_site/_ro/trn_rl_repo",
                  "/root/.axon_site/_ro/pypackages", "/opt/trn_rl_repo"):
            if os.path.isdir(p) and p not in sys.path:
                sys.path.append(p)


_ensure_concourse()

import concourse.bass as bass  # noqa: E402
import concourse.tile as tile  # noqa: E402
from concourse import bacc, mybir  # noqa: E402
from concourse import bass_utils  # noqa: E402
from contextlib import ExitStack  # noqa: E402

F32 = mybir.dt.float32
BF16 = mybir.dt.bfloat16
EXP = mybir.ActivationFunctionType.Exp

S = 2048      # sequence length
M = 1024      # d_model
DH = 64       # d_head
HL = 4        # heads per core
NP = 2        # head pairs per core
CH = 512      # q-chunk width
NCH = S // CH     # 4 q chunks
KB = S // 128     # 16 k blocks
MB = M // 128     # 8 m blocks
N_CORES = 8
WARMUP = 34   # dummy N=128 matmuls at t=0 to keep HAM warm through DMA wait


def _emit(tc, nc, d, zero_bias):
    mm = nc.tensor.matmul
    with ExitStack() as ctx:
        # ---- persistent pools ----
        qkp = ctx.enter_context(tc.tile_pool(name="qkp", bufs=1))
        vp = ctx.enter_context(tc.tile_pool(name="vp", bufs=1))
        wop = ctx.enter_context(tc.tile_pool(name="wop", bufs=1))
        cst = ctx.enter_context(tc.tile_pool(name="cst", bufs=1))
        z2p = ctx.enter_context(tc.tile_pool(name="z2p", bufs=1))
        xp = ctx.enter_context(tc.tile_pool(name="xp", bufs=1))
        wp = ctx.enter_context(tc.tile_pool(name="wp", bufs=1))
        pp = ctx.enter_context(tc.tile_pool(name="pp", bufs=6))
        op_sb = ctx.enter_context(tc.tile_pool(name="op_sb", bufs=4))
        stg = ctx.enter_context(tc.tile_pool(name="stg", bufs=2))
        # PSUM budget is 8 banks, statically split: psS 2x2, psZ 2x1, and a
        # single 2-slot pool shared by every 1-bank psum use
        psX = ctx.enter_context(tc.tile_pool(name="psX", bufs=2, space="PSUM"))
        psS = ctx.enter_context(tc.tile_pool(name="psS", bufs=2, space="PSUM"))
        psZ = ctx.enter_context(tc.tile_pool(name="psZ", bufs=1, space="PSUM"))

        qt = [qkp.tile([128, S], BF16, name=f"qt{p}") for p in range(NP)]
        kt = [qkp.tile([128, S], BF16, name=f"kt{p}") for p in range(NP)]
        vt = [vp.tile([128, 260], BF16, name=f"vt{j}") for j in range(KB)]
        wo_t = [wop.tile([128, M], BF16, name=f"wo{p}") for p in range(NP)]
        dtri = cst.tile([128, 256], BF16, name="dtri")
        e33 = cst.tile([97, 128], BF16, name="e33")
        wu = cst.tile([128, 128], BF16, name="wu")
        # per-pair reciprocal denominators, bf16, at partitions 64 (half 0)
        # and 96 (half 1); rows 65..95 stay zero so the e33 broadcast matmul
        # sees finite values under its zero weights
        rdcb = cst.tile([97, NP * CH], BF16, name="rdcb")
        # reciprocal staging lives entirely at partition 0: the custom DVE
        # reciprocal op only behaves correctly with zero partition offset.
        # Column layout: [p0h0 | p0h1 | p1h0 | p1h1], 512 each.
        rdcf_a = cst.tile([1, NP * 2 * CH], F32, name="rdcf_a")
        rdcf_b = cst.tile([1, NP * 2 * CH], F32, name="rdcf_b")
        # per-(pair,half) suffix V column-sums: partition d (base 0), col =
        # chunk index. Computed on device by projecting host-side suffix
        # sums of x through W_V (plus counts*b_V via the ones-row trick).
        # Kept per-half at base partition 0: scalar_tensor_tensor requires
        # equal base partitions for its SBUF inputs.
        sfxT = [[cst.tile([64, 4], F32, name=f"sfxT{p}_{h}") for h in range(2)]
                for p in range(NP)]
        xsum_t = xp.tile([128, 4 * MB], BF16, name="xsum")
        if not zero_bias:
            xsum_ones = xp.tile([1, 4], BF16, name="xsum_ones")
        z2u = [z2p.tile([128, S], BF16, name=f"z2u{p}") for p in range(NP)]

        xt_t = [xp.tile([128, S], BF16, name=f"xt{mb}") for mb in range(MB)]
        wq_t = [wp.tile([128, 256], BF16, name=f"wq{mb}") for mb in range(MB)]
        wk_t = [wp.tile([128, 256], BF16, name=f"wk{mb}") for mb in range(MB)]
        wv_t = [wp.tile([128, 256], BF16, name=f"wv{mb}") for mb in range(MB)]
        if not zero_bias:
            xt_ones = xp.tile([1, S], BF16, name="xt_ones")
            wq_b = wp.tile([1, 256], BF16, name="wq_b")
            wk_b = wp.tile([1, 256], BF16, name="wk_b")
            wv_b = wp.tile([1, 256], BF16, name="wv_b")

        # ---- PE warm-up: dummy matmuls on a zeroed tile keep the PE busy
        # (and the HAM clock-gate open) while the first weight DMAs land ----
        nc.vector.memset(wu[:], 0.0)
        nc.vector.memset(e33[64:97, :], 0.0)
        nc.vector.memset(e33[64:65, 0:64], 1.0)
        nc.vector.memset(e33[96:97, 64:128], 1.0)
        nc.vector.memset(rdcb[64:97, :], 0.0)
        for w in range(WARMUP):
            wps = psX.tile([128, 128], F32, name="wps", tag="px")
            mm(wps[:], wu[:], wu[:], start=True, stop=True)

        # DMA order: what attention chunk 0 needs first (wq/wk, x^T chunk 0,
        # wv, masks), then the rest of x^T; W_O last. Intro loads are split
        # across both HWDGE dispatch engines (sync + scalar) since the
        # scalar engine is idle until the first exp.
        for mb in range(MB):
            ea = nc.scalar if mb % 2 else nc.sync
            eb = nc.sync if mb % 2 else nc.scalar
            ea.dma_start(wq_t[mb][:], d["wq"][128 * mb:128 * (mb + 1), :])
            if mb < 2:
                # first tiles gate the first matmuls: halve their latency by
                # loading each on two queues
                eb.dma_start(xt_t[mb][:, 0:256],
                             d["xt"][128 * mb:128 * (mb + 1), 0:256])
                ea.dma_start(xt_t[mb][:, 256:CH],
                             d["xt"][128 * mb:128 * (mb + 1), 256:CH])
            else:
                eb.dma_start(xt_t[mb][:, 0:CH],
                             d["xt"][128 * mb:128 * (mb + 1), 0:CH])
            ea.dma_start(wk_t[mb][:], d["wk"][128 * mb:128 * (mb + 1), :])
        for mb in range(MB):
            eng = nc.sync if mb % 2 else nc.scalar
            eng.dma_start(wv_t[mb][:], d["wv"][128 * mb:128 * (mb + 1), :])
        if not zero_bias:
            nc.sync.dma_start(wq_b[:], d["wq"][1024:1025, :])
            nc.sync.dma_start(wk_b[:], d["wk"][1024:1025, :])
            nc.sync.dma_start(wv_b[:], d["wv"][1024:1025, :])
            nc.sync.dma_start(xt_ones[:], d["xt"][1024:1025, :])
        nc.scalar.dma_start(dtri[:], d["mk"][:])
        nc.scalar.dma_start(
            xsum_t.rearrange("p (m c) -> p m c", c=4),
            d["xs"][0:1024, :].rearrange("(m p) c -> p m c", p=128))
        if not zero_bias:
            nc.scalar.dma_start(xsum_ones[:], d["xs"][1024:1025, :])
        for c in range(1, NCH):
            for mb in range(MB):
                nc.sync.dma_start(
                    xt_t[mb][:, CH * c:CH * (c + 1)],
                    d["xt"][128 * mb:128 * (mb + 1), CH * c:CH * (c + 1)])
        for p in range(NP):
            nc.sync.dma_start(wo_t[p][:], d["wo"][128 * p:128 * (p + 1), :])

        def emit_v(j0):
            # two k-blocks share one [128,512] psum bank: halves the shared
            # psum slot churn. V is packed head-major (256 cols); the
            # per-head ones columns of vt are memset separately.
            ps = psX.tile([128, 2 * 256], F32, name="psv", tag="px")
            for dj in range(2):
                j = j0 + dj
                for mb in range(MB):
                    mm(ps[:, 256 * dj:256 * (dj + 1)],
                       xt_t[mb][:, 128 * j:128 * (j + 1)],
                       wv_t[mb][:], start=(mb == 0),
                       stop=(zero_bias and mb == MB - 1))
                if not zero_bias:
                    mm(ps[:, 256 * dj:256 * (dj + 1)],
                       xt_ones[:, 128 * j:128 * (j + 1)], wv_b[:],
                       start=False, stop=True)
            for dj in range(2):
                j = j0 + dj
                pssl = ps[:, 256 * dj:256 * (dj + 1)]
                nc.vector.tensor_copy(
                    vt[j].rearrange("p (h c) -> p h c", c=65)[:, :, 0:64],
                    pssl.rearrange("p (h c) -> p h c", c=64))
                oc = vt[j].rearrange("p (h c) -> p h c", c=65)[:, :, 64]
                nc.gpsimd.memset(oc, 1.0)

        def emit_qk(p, which, c):
            dst, wt = (qt, wq_t) if which == 0 else (kt, wk_t)
            ps = psX.tile([128, CH], F32, name="psqk", tag="px")
            for mb in range(MB):
                mm(ps[:], wt[mb][:, 128 * p:128 * (p + 1)],
                   xt_t[mb][:, CH * c:CH * (c + 1)],
                   start=(mb == 0), stop=(zero_bias and mb == MB - 1))
            if not zero_bias:
                wb = wq_b if which == 0 else wk_b
                mm(ps[:], wb[:, 128 * p:128 * (p + 1)],
                   xt_ones[:, CH * c:CH * (c + 1)], start=False, stop=True)
            nc.vector.tensor_copy(dst[p][:, CH * c:CH * (c + 1)], ps[:])

        zps_by_pair = {}

        def emit_attn_pair(ch, p):
            # scores/mask/exp/AV for all k-blocks of chunk ch, head pair p
            nj = 4 * ch + 4
            if True:
                h0, h1 = 2 * p, 2 * p + 1
                zps = [psZ.tile([65, CH], F32, name=f"zps{half}",
                                tag=f"zps{half}") for half in range(2)]
                zps_by_pair[(ch, p)] = zps
                for j in range(nj):
                    # both heads of the pair share one [128,1024] scores
                    # psum: one mask-mul + one exp per j. For diagonal
                    # blocks only the unmasked column suffix [w0:512) is
                    # computed; the fully-masked prefix of P is exp(0)=1
                    # exactly, so it is memset to 1.0 (GPSIMD) instead.
                    r = j - 4 * ch
                    w0 = 128 * r if r > 0 else 0
                    last = j == nj - 1
                    sps = psS.tile([128, 2 * CH], F32, name="sps", tag="sps")
                    mm(sps[:, w0:CH],
                       kt[p][0:64, 128 * j:128 * (j + 1)],
                       qt[p][0:64, CH * ch + w0:CH * (ch + 1)],
                       start=True, stop=True)
                    mm(sps[:, CH + w0:2 * CH],
                       kt[p][64:128, 128 * j:128 * (j + 1)],
                       qt[p][64:128, CH * ch + w0:CH * (ch + 1)],
                       start=True, stop=True)
                    sps3 = sps.rearrange("p (t c) -> p t c", t=2)
                    pt = pp.tile([128, 2 * CH], BF16, name="pt", tag="pt")
                    pt3 = pt.rearrange("p (t c) -> p t c", t=2)
                    if r >= 0:
                        # triangular mask on the 128-wide diagonal strip of
                        # both heads at once
                        strip = sps3[:, :, w0:w0 + 128]
                        dtri3 = dtri.rearrange("p (t c) -> p t c", t=2)
                        nc.vector.tensor_mul(strip, strip, dtri3)
                    if w0:
                        nc.gpsimd.memset(pt3[:, :, 0:w0], 1.0)
                        nc.scalar.activation(pt3[:, :, w0:CH],
                                             sps3[:, :, w0:CH], EXP,
                                             scale=0.125)
                    else:
                        nc.scalar.activation(pt[:], sps[:], EXP, scale=0.125)
                    mm(zps[0][:], vt[j][:, 65 * h0:65 * h0 + 65],
                       pt[:, 0:CH], start=(j == 0), stop=last)
                    mm(zps[1][:], vt[j][:, 65 * h1:65 * h1 + 65],
                       pt[:, CH:2 * CH], start=(j == 0), stop=last)

        def emit_sfx():
            # project host-side suffix sums of x through W_V: per pair a
            # [128,4] psum with partition hb+d, col = chunk index, holding
            # sum_{k in suffix(ch)} V[k, d] (+ count*b_V via the ones row)
            for p in range(NP):
                ps = psX.tile([128, 4], F32, name="pssfx", tag="px")
                for mb in range(MB):
                    mm(ps[:], wv_t[mb][:, 128 * p:128 * (p + 1)],
                       xsum_t[:, 4 * mb:4 * (mb + 1)],
                       start=(mb == 0), stop=(zero_bias and mb == MB - 1))
                if not zero_bias:
                    mm(ps[:], wv_b[:, 128 * p:128 * (p + 1)],
                       xsum_ones[:], start=False, stop=True)
                for half in range(2):
                    nc.vector.tensor_copy(sfxT[p][half][:],
                                          ps[64 * half:64 * half + 64, :])

        stg_by_pair = {}

        def emit_zstage(ch, p, tail=False):
            # release-critical: one [65,512] PSUM->SBUF copy per half frees
            # the z psum banks; everything else (reciprocal, broadcast,
            # divide) reads the SBUF staging copy later, off this path.
            # At the tail the scalar engine is idle (no more exps), so the
            # copies run there instead of the loaded vector engine.
            zl = zps_by_pair.pop((ch, p))
            sg = [stg.tile([65, CH], F32, name=f"sg{half}", tag=f"sg{half}")
                  for half in range(2)]
            for half in range(2):
                if tail:
                    nc.scalar.activation(sg[half][:], zl[half][:],
                                         mybir.ActivationFunctionType.Copy)
                else:
                    nc.vector.tensor_copy(sg[half][:], zl[half][:])
            stg_by_pair[(ch, p)] = sg

        def emit_zdiv(ch, p, tail=False):
            # deferred division for (chunk, pair): denominator (+ suffix
            # count) -> reciprocal at partition 0 -> bf16 rows 64/96 ->
            # one 33-row broadcast matmul -> fused (z + sfx) * (1/d) into
            # z2u. Fully-masked suffix k-blocks contribute P=1 each: a
            # count to the denominator and sfxT to z.
            sg = stg_by_pair.pop((ch, p))
            cnt = float(S - CH * (ch + 1))
            rsl = rdcb[:, CH * p:CH * (p + 1)]
            ra = rdcf_a[:, 2 * CH * p:2 * CH * (p + 1)]
            rb = rdcf_b[:, 2 * CH * p:2 * CH * (p + 1)]
            for half in range(2):
                if tail:
                    nc.scalar.activation(ra[0:1, CH * half:CH * (half + 1)],
                                         sg[half][64:65, :],
                                         mybir.ActivationFunctionType.Copy,
                                         bias=cnt)
                else:
                    nc.vector.tensor_scalar_add(
                        ra[0:1, CH * half:CH * (half + 1)],
                        sg[half][64:65, :], cnt)
            nc.vector.reciprocal_approx_fast(rb[0:1, :], ra[0:1, :])
            for half, row in ((0, 64), (1, 96)):
                nc.vector.tensor_copy(rsl[row:row + 1, :],
                                      rb[0:1, CH * half:CH * (half + 1)])
            bc = psX.tile([128, CH], F32, name="bc", tag="px")
            mm(bc[:], e33[64:97, :], rsl[64:97, :], start=True, stop=True)
            for half in range(2):
                hb = 64 * half
                dst = z2u[p][hb:hb + 64, CH * ch:CH * (ch + 1)]
                if cnt:
                    nc.vector.scalar_tensor_tensor(
                        dst, sg[half][0:64, :],
                        sfxT[p][half][:, ch:ch + 1], bc[hb:hb + 64, :],
                        mybir.AluOpType.add, mybir.AluOpType.mult)
                else:
                    nc.vector.tensor_mul(dst, sg[half][0:64, :],
                                         bc[hb:hb + 64, :])

        def emit_divE(ch, tail=False):
            # project chunk ch's z to the output and stream to DRAM. Out
            # DMAs ride the sync queue (dispatching them on the scalar
            # queue delays exp, which gates attention); at the tail the
            # scalar engine+queue are idle, so half the casts and DMAs
            # move there to halve the drain.
            for q in range(4 * ch, 4 * ch + 4):
                for mc in range(2):
                    ops = psX.tile([128, CH], F32, name="ops", tag="px")
                    for p in range(NP):
                        mm(ops[:], z2u[p][:, 128 * q:128 * (q + 1)],
                           wo_t[p][:, CH * mc:CH * (mc + 1)],
                           start=(p == 0), stop=(p == 1))
                    osb = op_sb.tile([128, CH], BF16, name="osb", tag="osb")
                    if tail and mc:
                        nc.scalar.activation(
                            osb[:], ops[:], mybir.ActivationFunctionType.Copy)
                    else:
                        nc.vector.tensor_copy(osb[:], ops[:])
                    eng = nc.scalar if (tail and mc) else nc.sync
                    eng.dma_start(
                        d["out"][128 * q:128 * (q + 1), CH * mc:CH * (mc + 1)],
                        osb[:])

        # ---- emission: emission order doubles as scheduler priority.
        # Per chunk: attention pair 0, its release copies, then the
        # previous chunk's deferred division for pair 0 (fills DVE during
        # this chunk's attention); same for pair 1; then next chunk's Q/K
        # (PE filler at the boundary) and the previous chunk's output
        # projection. The division chains and O-proj run a full pair/chunk
        # behind the attention that produced their data, so the z psum
        # handoff between pairs only ever waits on the two staging copies.
        for p in range(NP):
            emit_qk(p, 0, 0)
            emit_qk(p, 1, 0)
        for j in range(0, 4, 2):
            emit_v(j)
        emit_attn_pair(0, 0)
        for j in range(4, KB, 2):
            emit_v(j)
        emit_sfx()
        emit_zstage(0, 0)
        emit_attn_pair(0, 1)
        emit_zstage(0, 1)
        for ch in range(1, NCH):
            for p in range(NP):
                emit_qk(p, 0, ch)
                emit_qk(p, 1, ch)
            emit_attn_pair(ch, 0)
            emit_zstage(ch, 0)
            emit_zdiv(ch - 1, 0)
            emit_attn_pair(ch, 1)
            emit_zstage(ch, 1, tail=(ch == 3))
            emit_zdiv(ch - 1, 1)
            emit_divE(ch - 1)
        emit_zdiv(3, 0)
        emit_zdiv(3, 1, tail=True)
        emit_divE(3, tail=True)


def build_program(zero_bias=False):
    nc = bacc.Bacc("TRN2", target_bir_lowering=False, debug=False,
                   num_devices=N_CORES)
    d = {
        "xt": nc.dram_tensor("xt", [1025, S], BF16, kind="ExternalInput").ap(),
        "wq": nc.dram_tensor("wq", [1025, 256], BF16, kind="ExternalInput").ap(),
        "wk": nc.dram_tensor("wk", [1025, 256], BF16, kind="ExternalInput").ap(),
        "wv": nc.dram_tensor("wv", [1025, 256], BF16, kind="ExternalInput").ap(),
        "wo": nc.dram_tensor("wo", [256, M], BF16, kind="ExternalInput").ap(),
        "mk": nc.dram_tensor("mk", [128, 256], BF16, kind="ExternalInput").ap(),
        "xs": nc.dram_tensor("xs", [1025, 4], BF16, kind="ExternalInput").ap(),
        "out": nc.dram_tensor("out", [S, M], BF16, kind="ExternalOutput").ap(),
    }
    with tile.TileContext(nc) as tc:
        _emit(tc, nc, d, zero_bias)
    nc.compile()
    return nc


_CACHE = {}


def _get_program(zero_bias=False):
    key = ("nc", zero_bias)
    if key not in _CACHE:
        _CACHE[key] = build_program(zero_bias)
    return _CACHE[key]


def _pack_qk(w4, b4):
    # w4 [4,1024,64], b4 [4,64] -> [1025, 256] (m-major, head-major cols)
    r = np.empty((1025, 256), np.float32)
    r[:1024] = w4.transpose(1, 0, 2).reshape(1024, 256)
    r[1024] = b4.reshape(256)
    return r


def _pack_v(w4, b4):
    # [1025, 256] head-major; vt ones columns are memset on device
    r = np.empty((1025, 256), np.float32)
    r[:1024] = w4.transpose(1, 0, 2).reshape(1024, 256)
    r[1024] = b4.reshape(256)
    return r


def prepare_in_maps(normalized_resid_pre, W_Q, b_Q, W_K, b_K, W_V, b_V, W_O,
                    b_O):
    import ml_dtypes
    bf16 = ml_dtypes.bfloat16
    x = np.asarray(normalized_resid_pre, np.float32)
    W_Q = np.asarray(W_Q, np.float32)
    b_Q = np.asarray(b_Q, np.float32)
    W_K = np.asarray(W_K, np.float32)
    b_K = np.asarray(b_K, np.float32)
    W_V = np.asarray(W_V, np.float32)
    b_V = np.asarray(b_V, np.float32)
    W_O = np.asarray(W_O, np.float32)

    tri = np.triu(np.ones((128, 128), np.float32))  # [k,q]: 1 where k <= q
    mk = np.tile(tri, (1, 2))  # both heads of a pair side by side

    xts = []
    xss = []
    for b in range(2):
        xt = np.empty((1025, S), np.float32)
        xt[:1024] = x[b].T
        xt[1024] = 1.0
        xts.append(xt.astype(bf16))
        # suffix sums of x over k >= 512*(c+1) (input preprocessing for the
        # on-device fully-masked-suffix correction), plus suffix counts in
        # the ones row so the bias matmul picks up count*b_V
        xs = np.zeros((1025, 4), np.float32)
        for c in range(3):
            xs[:1024, c] = x[b][512 * (c + 1):].sum(axis=0)
            xs[1024, c] = S - 512 * (c + 1)
        xss.append(xs.astype(bf16))

    in_maps = []
    for c in range(N_CORES):
        b, g = divmod(c, 4)
        hs = slice(4 * g, 4 * g + 4)
        in_maps.append({
            "xt": xts[b],
            "wq": _pack_qk(W_Q[hs], b_Q[hs]).astype(bf16),
            "wk": _pack_qk(W_K[hs], b_K[hs]).astype(bf16),
            "wv": _pack_v(W_V[hs], b_V[hs]).astype(bf16),
            "wo": np.ascontiguousarray(W_O[hs].reshape(256, M)).astype(bf16),
            "mk": mk.astype(bf16),
            "xs": xss[b],
        })
    return in_maps


def gather(results, b_O):
    out = np.zeros((2, S, M), np.float32)
    for c in range(N_CORES):
        out[c // 4] += np.asarray(results[c]["out"], dtype=np.float32)
    out += np.asarray(b_O, np.float32)[None, None, :]
    return out


def _run(in_maps, trace=False, zero_bias=False, **kw):
    nc = _get_program(zero_bias)
    return bass_utils.run_bass_kernel_spmd(
        nc, in_maps, core_ids=list(range(N_CORES)), trace=trace, **kw)


def all_zero_bias(b_Q, b_K, b_V):
    return (not np.any(np.asarray(b_Q)) and not np.any(np.asarray(b_K))
            and not np.any(np.asarray(b_V)))


def kernel(normalized_resid_pre, W_Q, b_Q, W_K, b_K, W_V, b_V, W_O, b_O):
    in_maps = prepare_in_maps(normalized_resid_pre, W_Q, b_Q, W_K, b_K, W_V,
                              b_V, W_O, b_O)
    res = _run(in_maps, zero_bias=all_zero_bias(b_Q, b_K, b_V))
    return gather(res.results, b_O)


# revision 71
# speedup vs baseline: 1.0389x; 1.0313x over previous
"""Trainium2 Bass kernel for nn_Attention_46780783788294.

Multi-head causal-ish attention (mask fills with 0.0, not -inf) for
x:[2,2048,1024], 16 heads of d_head=64, fp32 in/out, bf16 compute.

Sharding: 8 cores = 2 batches x 4 head-groups (4 heads each). Each core
computes its batch/head-group partial output [2048,1024] (fp32); host sums
the 4 partials per batch and adds b_O.

Per-core device program (all-transposed "S^T" layout, bf16 matmuls with
fp32 PSUM accumulation):
  xT_aug [1025,2048] (x^T plus ones row) and packed/augmented weights come
  from the host. QT/KT computed per head-pair [128,2048] (d on partitions);
  V computed in natural [k,d] layout [128,260] per k-block with a per-head
  ones column (from the bias-row trick) so the AV matmul accumulates the
  softmax denominator for free. Scores are built k-on-partitions so
  P = exp(mask * s / 8) feeds the AV matmul directly with no transposes;
  masked entries give exp(0)=1 exactly as the reference's 0.0-fill softmax
  requires. Fully-masked regions of P (the column prefix of diagonal
  k-blocks) are not computed at all: P=1 there, so the prefix is memset to
  1.0 on the (otherwise idle) GPSIMD engine and the AV matmul runs full
  width — no analytic-correction matmuls needed. The two heads of a pair
  share one [128,1024] scores psum so each mask-mul/exp covers both heads
  in one instruction; the two 64-contraction score matmuls row-pack into
  disjoint halves of the PE array and run concurrently.

  Fully-masked suffix k-blocks (all of P = 1 there) are handled with zero
  attention work: the host ships 3 suffix-sum columns of x, the device
  projects them through W_V (16 tiny matmuls), and the result is added
  during the division; the denominator's suffix contribution is a
  compile-time count constant.

  Softmax denominators ride in the AV psum at partition 64. Each (chunk,
  pair) ends in a release-critical stage (two [65,512] psum->sbuf copies
  that free the z psum banks for the next pair) followed by a deferred
  division chain (denominator +count -> reciprocal at partition 0 ->
  bf16 rows 0/32 -> per-half 1-row broadcast matmuls -> fused
  (z + sfx) * (1/d) into z2u) that the scheduler sinks into the following
  pair's attention window. The custom-DVE reciprocal op only works with
  SBUF operands at partition offset 0, hence the staging layout.

  Chunk-outer loop: each q-chunk is attended, divided, output-projected
  and DMA'd out before the next, so PE/ACT/DVE/DMA stay overlapped. Out
  DMAs ride the sync queue only (scalar-queue dispatch would delay exp,
  which rate-limits the attention inner loop); at the tail the idle
  scalar engine takes the staging copies, half the output casts and half
  the output DMAs. A burst of dummy warm-up matmuls at t=0 keeps the PE
  clock un-throttled (HAM) through the initial weight-DMA wait.
"""

import os
import sys

import numpy as np


def _ensure_concourse():
    try:
        import concourse  # noqa: F401
    except ImportError:
        for p in ("/root/.axon_site", "/root/.axon_site/_ro/trn_rl_repo",
                  "/root/.axon_site/_ro/pypackages", "/opt/trn_rl_repo"):
            if os.path.isdir(p) and p not in sys.path:
                sys.path.append(p)


_ensure_concourse()

import concourse.bass as bass  # noqa: E402
import concourse.tile as tile  # noqa: E402
from concourse import bacc, mybir  # noqa: E402
from concourse import bass_utils  # noqa: E402
from contextlib import ExitStack  # noqa: E402

F32 = mybir.dt.float32
BF16 = mybir.dt.bfloat16
EXP = mybir.ActivationFunctionType.Exp

S = 2048      # sequence length
M = 1024      # d_model
DH = 64       # d_head
HL = 4        # heads per core
NP = 2        # head pairs per core
CH = 512      # q-chunk width
NCH = S // CH     # 4 q chunks
KB = S // 128     # 16 k blocks
MB = M // 128     # 8 m blocks
N_CORES = 8
WARMUP = 12   # dummy N=128 matmuls at t=0 to keep HAM warm through DMA wait


def _emit(tc, nc, d, zero_bias):
    mm = nc.tensor.matmul
    with ExitStack() as ctx:
        # ---- persistent pools ----
        qkp = ctx.enter_context(tc.tile_pool(name="qkp", bufs=1))
        vp = ctx.enter_context(tc.tile_pool(name="vp", bufs=1))
        wop = ctx.enter_context(tc.tile_pool(name="wop", bufs=1))
        cst = ctx.enter_context(tc.tile_pool(name="cst", bufs=1))
        z2p = ctx.enter_context(tc.tile_pool(name="z2p", bufs=1))
        xp = ctx.enter_context(tc.tile_pool(name="xp", bufs=1))
        wp = ctx.enter_context(tc.tile_pool(name="wp", bufs=1))
        pp = ctx.enter_context(tc.tile_pool(name="pp", bufs=10))
        op_sb = ctx.enter_context(tc.tile_pool(name="op_sb", bufs=8))
        stg = ctx.enter_context(tc.tile_pool(name="stg", bufs=3))
        # PSUM budget is 8 banks, statically split: psS 2x2, psZ 2x1, and a
        # single 2-slot pool shared by every 1-bank psum use
        psX = ctx.enter_context(tc.tile_pool(name="psX", bufs=2, space="PSUM"))
        psS = ctx.enter_context(tc.tile_pool(name="psS", bufs=2, space="PSUM"))
        psZ = ctx.enter_context(tc.tile_pool(name="psZ", bufs=1, space="PSUM"))

        qt = [qkp.tile([128, S], BF16, name=f"qt{p}") for p in range(NP)]
        kt = [qkp.tile([128, S], BF16, name=f"kt{p}") for p in range(NP)]
        vt = [vp.tile([128, 260], BF16, name=f"vt{j}") for j in range(KB)]
        wo_t = [wop.tile([128, M], BF16, name=f"wo{p}") for p in range(NP)]
        dtri = cst.tile([128, 256], BF16, name="dtri")
        e33 = cst.tile([33, 128], BF16, name="e33")
        # per-pair reciprocal denominators, bf16, at partitions 0 (half 0)
        # and 32 (half 1); each per-half broadcast matmul reads exactly its
        # one row
        rdcb = cst.tile([33, NP * CH], BF16, name="rdcb")
        # reciprocal staging lives entirely at partition 0: the custom DVE
        # reciprocal op only behaves correctly with zero partition offset.
        # Column layout: [p0h0 | p0h1 | p1h0 | p1h1], 512 each.
        rdcf_a = cst.tile([1, NP * 2 * CH], F32, name="rdcf_a")
        rdcf_b = cst.tile([1, NP * 2 * CH], F32, name="rdcf_b")
        # dedicated staging for the tail pair's denominators: the shared
        # rdcf_a region is still owned by the previous chunk's deferred
        # division at that point in the program
        rdcf_t = cst.tile([1, 2 * CH], F32, name="rdcf_t")
        # per-(pair,half) suffix V column-sums: partition d (base 0), col =
        # chunk index. Computed on device by projecting host-side suffix
        # sums of x through W_V (plus counts*b_V via the ones-row trick).
        # Kept per-half at base partition 0: scalar_tensor_tensor requires
        # equal base partitions for its SBUF inputs.
        sfxT = [[cst.tile([64, 4], F32, name=f"sfxT{p}_{h}") for h in range(2)]
                for p in range(NP)]
        xsum_t = xp.tile([128, 4 * MB], BF16, name="xsum")
        if not zero_bias:
            xsum_ones = xp.tile([1, 4], BF16, name="xsum_ones")
        z2u = [z2p.tile([128, S], BF16, name=f"z2u{p}") for p in range(NP)]

        xt_t = [xp.tile([128, S], BF16, name=f"xt{mb}") for mb in range(MB)]
        # merged weight tiles: each projection's 8 m-block slices live in
        # one [128, 2048] tile loaded by a single DMA descriptor (the intro
        # is DMA-dispatch bound with per-block descriptors)
        wqm = wp.tile([128, 256 * MB], BF16, name="wqm")
        wkm = wp.tile([128, 256 * MB], BF16, name="wkm")
        wvm = wp.tile([128, 256 * MB], BF16, name="wvm")
        wq_t = [wqm[:, 256 * mb:256 * (mb + 1)] for mb in range(MB)]
        wk_t = [wkm[:, 256 * mb:256 * (mb + 1)] for mb in range(MB)]
        wv_t = [wvm[:, 256 * mb:256 * (mb + 1)] for mb in range(MB)]
        if not zero_bias:
            xt_ones = xp.tile([1, S], BF16, name="xt_ones")
            wq_b = wp.tile([1, 256], BF16, name="wq_b")
            wk_b = wp.tile([1, 256], BF16, name="wk_b")
            wv_b = wp.tile([1, 256], BF16, name="wv_b")

        # ---- PE warm-up: dummy matmuls on the (first-loaded) mask tile
        # keep the PE busy, and the HAM clock-gate open, while the weight
        # DMAs land. They use the scores-psum slots, which nothing needs
        # until attention starts, so they never gate the first qk matmuls.
        nc.vector.memset(e33[0:1, 0:64], 1.0)
        nc.vector.memset(e33[32:33, 64:128], 1.0)
        nc.scalar.dma_start(dtri[:], d["mk"][:])
        for w in range(WARMUP):
            wps = psS.tile([128, 128], F32, name="wps", tag="sps")
            mm(wps[:], dtri[:, 0:128], dtri[:, 0:128], start=True, stop=True)

        # DMA order: what attention chunk 0 needs first (wq/wk, x^T chunk 0,
        # wv, masks), then the rest of x^T; W_O last. Intro loads are split
        # across both HWDGE dispatch engines (sync + scalar) since the
        # scalar engine is idle until the first exp. Weights ride single
        # merged descriptors; x^T chunk 0 is two half descriptors, one per
        # queue, so the first qk accumulation chain starts ASAP.
        nc.sync.dma_start(wqm[:], d["wq2"][:])
        for mb in range(4):
            nc.scalar.dma_start(xt_t[mb][:, 0:CH],
                                d["xt"][128 * mb:128 * (mb + 1), 0:CH])
        for mb in range(4, MB):
            nc.sync.dma_start(xt_t[mb][:, 0:CH],
                              d["xt"][128 * mb:128 * (mb + 1), 0:CH])
        nc.scalar.dma_start(wkm[:], d["wk2"][:])
        nc.sync.dma_start(wvm[:], d["wv2"][:])
        if not zero_bias:
            nc.sync.dma_start(wq_b[:], d["wq"][1024:1025, :])
            nc.sync.dma_start(wk_b[:], d["wk"][1024:1025, :])
            nc.sync.dma_start(wv_b[:], d["wv"][1024:1025, :])
            nc.sync.dma_start(xt_ones[:], d["xt"][1024:1025, :])
        nc.scalar.dma_start(
            xsum_t.rearrange("p (m c) -> p m c", c=4),
            d["xs"][0:1024, :].rearrange("(m p) c -> p m c", p=128))
        if not zero_bias:
            nc.scalar.dma_start(xsum_ones[:], d["xs"][1024:1025, :])
        for c in range(1, NCH):
            for mb in range(MB):
                nc.sync.dma_start(
                    xt_t[mb][:, CH * c:CH * (c + 1)],
                    d["xt"][128 * mb:128 * (mb + 1), CH * c:CH * (c + 1)])
        for p in range(NP):
            nc.sync.dma_start(wo_t[p][:], d["wo"][128 * p:128 * (p + 1), :])

        def emit_v(j0):
            # two k-blocks share one [128,512] psum bank: halves the shared
            # psum slot churn. V is packed head-major (256 cols); the
            # per-head ones columns of vt are memset separately.
            ps = psX.tile([128, 2 * 256], F32, name="psv", tag="px")
            for dj in range(2):
                j = j0 + dj
                for mb in range(MB):
                    mm(ps[:, 256 * dj:256 * (dj + 1)],
                       xt_t[mb][:, 128 * j:128 * (j + 1)],
                       wv_t[mb][:], start=(mb == 0),
                       stop=(zero_bias and mb == MB - 1))
                if not zero_bias:
                    mm(ps[:, 256 * dj:256 * (dj + 1)],
                       xt_ones[:, 128 * j:128 * (j + 1)], wv_b[:],
                       start=False, stop=True)
            for dj in range(2):
                j = j0 + dj
                pssl = ps[:, 256 * dj:256 * (dj + 1)]
                nc.vector.tensor_copy(
                    vt[j].rearrange("p (h c) -> p h c", c=65)[:, :, 0:64],
                    pssl.rearrange("p (h c) -> p h c", c=64))
                oc = vt[j].rearrange("p (h c) -> p h c", c=65)[:, :, 64]
                nc.gpsimd.memset(oc, 1.0)

        def emit_qk(p, which, c):
            dst, wt = (qt, wq_t) if which == 0 else (kt, wk_t)
            ps = psX.tile([128, CH], F32, name="psqk", tag="px")
            for mb in range(MB):
                mm(ps[:], wt[mb][:, 128 * p:128 * (p + 1)],
                   xt_t[mb][:, CH * c:CH * (c + 1)],
                   start=(mb == 0), stop=(zero_bias and mb == MB - 1))
            if not zero_bias:
                wb = wq_b if which == 0 else wk_b
                mm(ps[:], wb[:, 128 * p:128 * (p + 1)],
                   xt_ones[:, CH * c:CH * (c + 1)], start=False, stop=True)
            nc.vector.tensor_copy(dst[p][:, CH * c:CH * (c + 1)], ps[:])

        zps_by_pair = {}

        def emit_attn_pair(ch, p):
            # scores/mask/exp/AV for all k-blocks of chunk ch, head pair p
            nj = 4 * ch + 4
            if True:
                h0, h1 = 2 * p, 2 * p + 1
                zps = [psZ.tile([65, CH], F32, name=f"zps{half}",
                                tag=f"zps{half}") for half in range(2)]
                zps_by_pair[(ch, p)] = zps
                for j in range(nj):
                    # both heads of the pair share one [128,1024] scores
                    # psum: one mask-mul + one exp per j. For diagonal
                    # blocks only the unmasked column suffix [w0:512) is
                    # computed; the fully-masked prefix of P is exp(0)=1
                    # exactly, so it is memset to 1.0 (GPSIMD) instead.
                    r = j - 4 * ch
                    w0 = 128 * r if r > 0 else 0
                    last = j == nj - 1
                    sps = psS.tile([128, 2 * CH], F32, name="sps", tag="sps")
                    mm(sps[:, w0:CH],
                       kt[p][0:64, 128 * j:128 * (j + 1)],
                       qt[p][0:64, CH * ch + w0:CH * (ch + 1)],
                       start=True, stop=True)
                    mm(sps[:, CH + w0:2 * CH],
                       kt[p][64:128, 128 * j:128 * (j + 1)],
                       qt[p][64:128, CH * ch + w0:CH * (ch + 1)],
                       start=True, stop=True)
                    sps3 = sps.rearrange("p (t c) -> p t c", t=2)
                    pt = pp.tile([128, 2 * CH], BF16, name="pt", tag="pt")
                    pt3 = pt.rearrange("p (t c) -> p t c", t=2)
                    if r >= 0:
                        # triangular mask on the 128-wide diagonal strip of
                        # both heads at once
                        strip = sps3[:, :, w0:w0 + 128]
                        dtri3 = dtri.rearrange("p (t c) -> p t c", t=2)
                        nc.vector.tensor_mul(strip, strip, dtri3)
                    if w0:
                        nc.gpsimd.memset(pt3[:, :, 0:w0], 1.0)
                        nc.scalar.activation(pt3[:, :, w0:CH],
                                             sps3[:, :, w0:CH], EXP,
                                             scale=0.125)
                    else:
                        nc.scalar.activation(pt[:], sps[:], EXP, scale=0.125)
                    mm(zps[0][:], vt[j][:, 65 * h0:65 * h0 + 65],
                       pt[:, 0:CH], start=(j == 0), stop=last)
                    mm(zps[1][:], vt[j][:, 65 * h1:65 * h1 + 65],
                       pt[:, CH:2 * CH], start=(j == 0), stop=last)

        def emit_sfx():
            # project host-side suffix sums of x through W_V: per pair a
            # [128,4] psum with partition hb+d, col = chunk index, holding
            # sum_{k in suffix(ch)} V[k, d] (+ count*b_V via the ones row)
            for p in range(NP):
                ps = psX.tile([128, 4], F32, name="pssfx", tag="px")
                for mb in range(MB):
                    mm(ps[:], wv_t[mb][:, 128 * p:128 * (p + 1)],
                       xsum_t[:, 4 * mb:4 * (mb + 1)],
                       start=(mb == 0), stop=(zero_bias and mb == MB - 1))
                if not zero_bias:
                    mm(ps[:], wv_b[:, 128 * p:128 * (p + 1)],
                       xsum_ones[:], start=False, stop=True)
                for half in range(2):
                    nc.vector.tensor_copy(sfxT[p][half][:],
                                          ps[64 * half:64 * half + 64, :])

        stg_by_pair = {}

        def emit_zstage(ch, p, tail=False):
            # release-critical: one [65,512] PSUM->SBUF copy per half frees
            # the z psum banks; everything else (reciprocal, broadcast,
            # divide) reads the SBUF staging copy later, off this path.
            # At the tail the scalar engine is idle (no more exps), so the
            # copies run there instead of the loaded vector engine.
            zl = zps_by_pair.pop((ch, p))
            sg = [stg.tile([65, CH], F32, name=f"sg{half}", tag=f"sg{half}")
                  for half in range(2)]
            if tail:
                # stage the denominators first (straight from PSUM, + the
                # suffix count) so the reciprocal chain starts immediately
                cnt = float(S - CH * (ch + 1))
                for half in range(2):
                    nc.scalar.activation(
                        rdcf_t[0:1, CH * half:CH * (half + 1)],
                        zl[half][64:65, :],
                        mybir.ActivationFunctionType.Copy, bias=cnt)
                for half in range(2):
                    nc.scalar.activation(sg[half][:], zl[half][:],
                                         mybir.ActivationFunctionType.Copy)
            else:
                for half in range(2):
                    nc.vector.tensor_copy(sg[half][:], zl[half][:])
            stg_by_pair[(ch, p)] = sg

        def emit_zdiv(ch, p, tail=False):
            # deferred division for (chunk, pair): denominator (+ suffix
            # count) -> reciprocal at partition 0 -> bf16 rows 64/96 ->
            # one 33-row broadcast matmul -> fused (z + sfx) * (1/d) into
            # z2u. Fully-masked suffix k-blocks contribute P=1 each: a
            # count to the denominator and sfxT to z.
            sg = stg_by_pair.pop((ch, p))
            cnt = float(S - CH * (ch + 1))
            rsl = rdcb[:, CH * p:CH * (p + 1)]
            ra = rdcf_t if tail else rdcf_a[:, 2 * CH * p:2 * CH * (p + 1)]
            rb = rdcf_b[:, 2 * CH * p:2 * CH * (p + 1)]
            bc = psX.tile([128, CH], F32, name="bc", tag="px")
            for half, row in ((0, 0), (1, 32)):
                hb = 64 * half
                if not tail:
                    # (at the tail the denominators were staged in zstage)
                    nc.vector.tensor_scalar_add(
                        ra[0:1, CH * half:CH * (half + 1)],
                        sg[half][64:65, :], cnt)
                # per-half reciprocal -> bf16 -> broadcast -> divide so
                # half 0's output is ready while half 1 still processes
                nc.vector.reciprocal_approx_fast(
                    rb[0:1, CH * half:CH * (half + 1)],
                    ra[0:1, CH * half:CH * (half + 1)])
                nc.vector.tensor_copy(rsl[row:row + 1, :],
                                      rb[0:1, CH * half:CH * (half + 1)])
                mm(bc[hb:hb + 64, :], e33[row:row + 1, 64 * half:64 * half + 64],
                   rsl[row:row + 1, :], start=True, stop=True)
                dst = z2u[p][hb:hb + 64, CH * ch:CH * (ch + 1)]
                if cnt:
                    nc.vector.scalar_tensor_tensor(
                        dst, sg[half][0:64, :],
                        sfxT[p][half][:, ch:ch + 1], bc[hb:hb + 64, :],
                        mybir.AluOpType.add, mybir.AluOpType.mult)
                else:
                    nc.vector.tensor_mul(dst, sg[half][0:64, :],
                                         bc[hb:hb + 64, :])

        def emit_divE(ch, tail=False):
            # project chunk ch's z to the output and stream to DRAM. Out
            # DMAs ride the sync queue (dispatching them on the scalar
            # queue delays exp, which gates attention); at the tail the
            # scalar engine+queue are idle, so half the casts and DMAs
            # move there to halve the drain.
            for q in range(4 * ch, 4 * ch + 4):
                for mc in range(2):
                    ops = psX.tile([128, CH], F32, name="ops", tag="px")
                    for p in range(NP):
                        mm(ops[:], z2u[p][:, 128 * q:128 * (q + 1)],
                           wo_t[p][:, CH * mc:CH * (mc + 1)],
                           start=(p == 0), stop=(p == 1))
                    osb = op_sb.tile([128, CH], BF16, name="osb", tag="osb")
                    if tail and mc:
                        nc.scalar.activation(
                            osb[:], ops[:], mybir.ActivationFunctionType.Copy)
                    else:
                        nc.vector.tensor_copy(osb[:], ops[:])
                    eng = nc.scalar if (tail and mc) else nc.sync
                    eng.dma_start(
                        d["out"][128 * q:128 * (q + 1), CH * mc:CH * (mc + 1)],
                        osb[:])

        # ---- emission: emission order doubles as scheduler priority.
        # Per chunk: attention pair 0, its release copies, then the
        # previous chunk's deferred division for pair 0 (fills DVE during
        # this chunk's attention); same for pair 1; then next chunk's Q/K
        # (PE filler at the boundary) and the previous chunk's output
        # projection. The division chains and O-proj run a full pair/chunk
        # behind the attention that produced their data, so the z psum
        # handoff between pairs only ever waits on the two staging copies.
        for p in range(NP):
            emit_qk(p, 0, 0)
            emit_qk(p, 1, 0)
        for j in range(0, 4, 2):
            emit_v(j)
        emit_attn_pair(0, 0)
        for j in range(4, KB, 2):
            emit_v(j)
        emit_sfx()
        emit_zstage(0, 0)
        emit_attn_pair(0, 1)
        emit_zstage(0, 1)
        for ch in range(1, NCH):
            for p in range(NP):
                emit_qk(p, 0, ch)
                emit_qk(p, 1, ch)
            emit_attn_pair(ch, 0)
            emit_zstage(ch, 0)
            emit_zdiv(ch - 1, 0)
            emit_attn_pair(ch, 1)
            emit_zstage(ch, 1, tail=(ch == 3))
            emit_zdiv(ch - 1, 1)
            emit_divE(ch - 1)
        emit_zdiv(3, 0)
        emit_zdiv(3, 1, tail=True)
        emit_divE(3, tail=True)


def build_program(zero_bias=False):
    nc = bacc.Bacc("TRN2", target_bir_lowering=False, debug=False,
                   num_devices=N_CORES)
    d = {
        "xt": nc.dram_tensor("xt", [1025, S], BF16, kind="ExternalInput").ap(),
        "wq": nc.dram_tensor("wq", [1025, 256], BF16, kind="ExternalInput").ap(),
        "wk": nc.dram_tensor("wk", [1025, 256], BF16, kind="ExternalInput").ap(),
        "wv": nc.dram_tensor("wv", [1025, 256], BF16, kind="ExternalInput").ap(),
        "wo": nc.dram_tensor("wo", [256, M], BF16, kind="ExternalInput").ap(),
        "wq2": nc.dram_tensor("wq2", [128, 2048], BF16, kind="ExternalInput").ap(),
        "wk2": nc.dram_tensor("wk2", [128, 2048], BF16, kind="ExternalInput").ap(),
        "wv2": nc.dram_tensor("wv2", [128, 2048], BF16, kind="ExternalInput").ap(),
        "mk": nc.dram_tensor("mk", [128, 256], BF16, kind="ExternalInput").ap(),
        "xs": nc.dram_tensor("xs", [1025, 4], BF16, kind="ExternalInput").ap(),
        "out": nc.dram_tensor("out", [S, M], BF16, kind="ExternalOutput").ap(),
    }
    with tile.TileContext(nc) as tc:
        _emit(tc, nc, d, zero_bias)
    nc.compile()
    return nc


_CACHE = {}


def _get_program(zero_bias=False):
    key = ("nc", zero_bias)
    if key not in _CACHE:
        _CACHE[key] = build_program(zero_bias)
    return _CACHE[key]


def _pack_qk(w4, b4):
    # w4 [4,1024,64], b4 [4,64] -> [1025, 256] (m-major, head-major cols)
    r = np.empty((1025, 256), np.float32)
    r[:1024] = w4.transpose(1, 0, 2).reshape(1024, 256)
    r[1024] = b4.reshape(256)
    return r


def _pack_v(w4, b4):
    # [1025, 256] head-major; vt ones columns are memset on device
    r = np.empty((1025, 256), np.float32)
    r[:1024] = w4.transpose(1, 0, 2).reshape(1024, 256)
    r[1024] = b4.reshape(256)
    return r


def prepare_in_maps(normalized_resid_pre, W_Q, b_Q, W_K, b_K, W_V, b_V, W_O,
                    b_O):
    import ml_dtypes
    bf16 = ml_dtypes.bfloat16
    x = np.asarray(normalized_resid_pre, np.float32)
    W_Q = np.asarray(W_Q, np.float32)
    b_Q = np.asarray(b_Q, np.float32)
    W_K = np.asarray(W_K, np.float32)
    b_K = np.asarray(b_K, np.float32)
    W_V = np.asarray(W_V, np.float32)
    b_V = np.asarray(b_V, np.float32)
    W_O = np.asarray(W_O, np.float32)

    tri = np.triu(np.ones((128, 128), np.float32))  # [k,q]: 1 where k <= q
    mk = np.tile(tri, (1, 2))  # both heads of a pair side by side

    xts = []
    xss = []
    for b in range(2):
        xt = np.empty((1025, S), np.float32)
        xt[:1024] = x[b].T
        xt[1024] = 1.0
        xts.append(xt.astype(bf16))
        # suffix sums of x over k >= 512*(c+1) (input preprocessing for the
        # on-device fully-masked-suffix correction), plus suffix counts in
        # the ones row so the bias matmul picks up count*b_V
        xs = np.zeros((1025, 4), np.float32)
        for c in range(3):
            xs[:1024, c] = x[b][512 * (c + 1):].sum(axis=0)
            xs[1024, c] = S - 512 * (c + 1)
        xss.append(xs.astype(bf16))

    def merge(w):
        # [1024, 256] -> [128, 8*256]: m-block-major columns so the whole
        # projection loads as one DMA descriptor
        return np.ascontiguousarray(
            w[:1024].reshape(8, 128, 256).transpose(1, 0, 2).reshape(128, 2048))

    in_maps = []
    for c in range(N_CORES):
        b, g = divmod(c, 4)
        hs = slice(4 * g, 4 * g + 4)
        wq = _pack_qk(W_Q[hs], b_Q[hs]).astype(bf16)
        wk = _pack_qk(W_K[hs], b_K[hs]).astype(bf16)
        wv = _pack_v(W_V[hs], b_V[hs]).astype(bf16)
        in_maps.append({
            "xt": xts[b],
            "wq": wq, "wk": wk, "wv": wv,
            "wq2": merge(wq), "wk2": merge(wk), "wv2": merge(wv),
            "wo": np.ascontiguousarray(W_O[hs].reshape(256, M)).astype(bf16),
            "mk": mk.astype(bf16),
            "xs": xss[b],
        })
    return in_maps


def gather(results, b_O):
    out = np.zeros((2, S, M), np.float32)
    for c in range(N_CORES):
        out[c // 4] += np.asarray(results[c]["out"], dtype=np.float32)
    out += np.asarray(b_O, np.float32)[None, None, :]
    return out


def _run(in_maps, trace=False, zero_bias=False, **kw):
    nc = _get_program(zero_bias)
    return bass_utils.run_bass_kernel_spmd(
        nc, in_maps, core_ids=list(range(N_CORES)), trace=trace, **kw)


def all_zero_bias(b_Q, b_K, b_V):
    return (not np.any(np.asarray(b_Q)) and not np.any(np.asarray(b_K))
            and not np.any(np.asarray(b_V)))


def kernel(normalized_resid_pre, W_Q, b_Q, W_K, b_K, W_V, b_V, W_O, b_O):
    in_maps = prepare_in_maps(normalized_resid_pre, W_Q, b_Q, W_K, b_K, W_V,
                              b_V, W_O, b_O)
    res = _run(in_maps, zero_bias=all_zero_bias(b_Q, b_K, b_V))
    return gather(res.results, b_O)


# revision 72
# speedup vs baseline: 1.0537x; 1.0143x over previous
"""Trainium2 Bass kernel for nn_Attention_46780783788294.

Multi-head causal-ish attention (mask fills with 0.0, not -inf) for
x:[2,2048,1024], 16 heads of d_head=64, fp32 in/out, bf16 compute.

Sharding: 8 cores = 2 batches x 4 head-groups (4 heads each). Each core
computes its batch/head-group partial output [2048,1024] (fp32); host sums
the 4 partials per batch and adds b_O.

Per-core device program (all-transposed "S^T" layout, bf16 matmuls with
fp32 PSUM accumulation):
  xT_aug [1025,2048] (x^T plus ones row) and packed/augmented weights come
  from the host. QT/KT computed per head-pair [128,2048] (d on partitions);
  V computed in natural [k,d] layout [128,260] per k-block with a per-head
  ones column (from the bias-row trick) so the AV matmul accumulates the
  softmax denominator for free. Scores are built k-on-partitions so
  P = exp(mask * s / 8) feeds the AV matmul directly with no transposes;
  masked entries give exp(0)=1 exactly as the reference's 0.0-fill softmax
  requires. Fully-masked regions of P (the column prefix of diagonal
  k-blocks) are not computed at all: P=1 there, so the prefix is memset to
  1.0 on the (otherwise idle) GPSIMD engine and the AV matmul runs full
  width — no analytic-correction matmuls needed. The two heads of a pair
  share one [128,1024] scores psum so each mask-mul/exp covers both heads
  in one instruction; the two 64-contraction score matmuls row-pack into
  disjoint halves of the PE array and run concurrently.

  Fully-masked suffix k-blocks (all of P = 1 there) are handled with zero
  attention work: the host ships 3 suffix-sum columns of x, the device
  projects them through W_V (16 tiny matmuls), and the result is added
  during the division; the denominator's suffix contribution is a
  compile-time count constant.

  Softmax denominators ride in the AV psum at partition 64. Each (chunk,
  pair) ends in a release-critical stage (two [65,512] psum->sbuf copies
  that free the z psum banks for the next pair) followed by a deferred
  division chain (denominator +count -> reciprocal at partition 0 ->
  bf16 rows 0/32 -> per-half 1-row broadcast matmuls -> fused
  (z + sfx) * (1/d) into z2u) that the scheduler sinks into the following
  pair's attention window. The custom-DVE reciprocal op only works with
  SBUF operands at partition offset 0, hence the staging layout.

  Chunk-outer loop: each q-chunk is attended, divided, output-projected
  and DMA'd out before the next, so PE/ACT/DVE/DMA stay overlapped. Out
  DMAs ride the sync queue only (scalar-queue dispatch would delay exp,
  which rate-limits the attention inner loop); at the tail the idle
  scalar engine takes the staging copies, half the output casts and half
  the output DMAs. A burst of dummy warm-up matmuls at t=0 keeps the PE
  clock un-throttled (HAM) through the initial weight-DMA wait.
"""

import os
import sys

import numpy as np


def _ensure_concourse():
    try:
        import concourse  # noqa: F401
    except ImportError:
        for p in ("/root/.axon_site", "/root/.axon_site/_ro/trn_rl_repo",
                  "/root/.axon_site/_ro/pypackages", "/opt/trn_rl_repo"):
            if os.path.isdir(p) and p not in sys.path:
                sys.path.append(p)


_ensure_concourse()

import concourse.bass as bass  # noqa: E402
import concourse.tile as tile  # noqa: E402
from concourse import bacc, mybir  # noqa: E402
from concourse import bass_utils  # noqa: E402
from contextlib import ExitStack  # noqa: E402

F32 = mybir.dt.float32
BF16 = mybir.dt.bfloat16
EXP = mybir.ActivationFunctionType.Exp

S = 2048      # sequence length
M = 1024      # d_model
DH = 64       # d_head
HL = 4        # heads per core
NP = 2        # head pairs per core
CH = 512      # q-chunk width
NCH = S // CH     # 4 q chunks
KB = S // 128     # 16 k blocks
MB = M // 128     # 8 m blocks
N_CORES = 8
WARMUP = 12   # dummy N=128 matmuls at t=0 to keep HAM warm through DMA wait


def _emit(tc, nc, d, zero_bias):
    mm = nc.tensor.matmul
    with ExitStack() as ctx:
        # ---- persistent pools ----
        qkp = ctx.enter_context(tc.tile_pool(name="qkp", bufs=1))
        vp = ctx.enter_context(tc.tile_pool(name="vp", bufs=1))
        wop = ctx.enter_context(tc.tile_pool(name="wop", bufs=1))
        cst = ctx.enter_context(tc.tile_pool(name="cst", bufs=1))
        z2p = ctx.enter_context(tc.tile_pool(name="z2p", bufs=1))
        xp = ctx.enter_context(tc.tile_pool(name="xp", bufs=1))
        wp = ctx.enter_context(tc.tile_pool(name="wp", bufs=1))
        pp = ctx.enter_context(tc.tile_pool(name="pp", bufs=8))
        op_sb = ctx.enter_context(tc.tile_pool(name="op_sb", bufs=6))
        stg = ctx.enter_context(tc.tile_pool(name="stg", bufs=2))
        # PSUM budget is 8 banks, statically split: psS 2x2, psZ 2x1, and a
        # single 2-slot pool shared by every 1-bank psum use
        psX = ctx.enter_context(tc.tile_pool(name="psX", bufs=2, space="PSUM"))
        psS = ctx.enter_context(tc.tile_pool(name="psS", bufs=2, space="PSUM"))
        psZ = ctx.enter_context(tc.tile_pool(name="psZ", bufs=1, space="PSUM"))

        qt = [qkp.tile([128, S], BF16, name=f"qt{p}") for p in range(NP)]
        kt = [qkp.tile([128, S], BF16, name=f"kt{p}") for p in range(NP)]
        vt = [vp.tile([128, 260], BF16, name=f"vt{j}") for j in range(KB)]
        wo_t = [wop.tile([128, M], BF16, name=f"wo{p}") for p in range(NP)]
        dtri = cst.tile([128, 256], BF16, name="dtri")
        e33 = cst.tile([33, 128], BF16, name="e33")
        # per-pair reciprocal denominators, bf16, at partitions 0 (half 0)
        # and 32 (half 1); each per-half broadcast matmul reads exactly its
        # one row
        rdcb = cst.tile([33, NP * CH], BF16, name="rdcb")
        # reciprocal staging lives entirely at partition 0: the custom DVE
        # reciprocal op only behaves correctly with zero partition offset.
        # Column layout: [p0h0 | p0h1 | p1h0 | p1h1], 512 each.
        rdcf_a = cst.tile([1, NP * 2 * CH], F32, name="rdcf_a")
        rdcf_b = cst.tile([1, NP * 2 * CH], F32, name="rdcf_b")
        # dedicated staging for the tail pair's denominators: the shared
        # rdcf_a region is still owned by the previous chunk's deferred
        # division at that point in the program
        rdcf_t = cst.tile([1, 2 * CH], F32, name="rdcf_t")
        # per-(pair,half) suffix V column-sums: partition d (base 0), col =
        # chunk index. Computed on device by projecting host-side suffix
        # sums of x through W_V (plus counts*b_V via the ones-row trick).
        # Kept per-half at base partition 0: scalar_tensor_tensor requires
        # equal base partitions for its SBUF inputs.
        sfxT = [[cst.tile([64, 4], F32, name=f"sfxT{p}_{h}") for h in range(2)]
                for p in range(NP)]
        xsum_t = xp.tile([128, 4 * MB], BF16, name="xsum")
        if not zero_bias:
            xsum_ones = xp.tile([1, 4], BF16, name="xsum_ones")
        z2u = [z2p.tile([128, S], BF16, name=f"z2u{p}") for p in range(NP)]

        xt_t = [xp.tile([128, S], BF16, name=f"xt{mb}") for mb in range(MB)]
        # merged weight tiles: each projection's 8 m-block slices live in
        # one [128, 2048] tile loaded by a single DMA descriptor (the intro
        # is DMA-dispatch bound with per-block descriptors)
        wqm = wp.tile([128, 256 * MB], BF16, name="wqm")
        wkm = wp.tile([128, 256 * MB], BF16, name="wkm")
        wvm = wp.tile([128, 256 * MB], BF16, name="wvm")
        wq_t = [wqm[:, 256 * mb:256 * (mb + 1)] for mb in range(MB)]
        wk_t = [wkm[:, 256 * mb:256 * (mb + 1)] for mb in range(MB)]
        wv_t = [wvm[:, 256 * mb:256 * (mb + 1)] for mb in range(MB)]
        if not zero_bias:
            xt_ones = xp.tile([1, S], BF16, name="xt_ones")
            wq_b = wp.tile([1, 256], BF16, name="wq_b")
            wk_b = wp.tile([1, 256], BF16, name="wk_b")
            wv_b = wp.tile([1, 256], BF16, name="wv_b")

        # ---- PE warm-up: dummy matmuls on the (first-loaded) mask tile
        # keep the PE busy, and the HAM clock-gate open, while the weight
        # DMAs land. They use the scores-psum slots, which nothing needs
        # until attention starts, so they never gate the first qk matmuls.
        nc.vector.memset(e33[0:1, 0:64], 1.0)
        nc.vector.memset(e33[32:33, 64:128], 1.0)
        nc.scalar.dma_start(dtri[:], d["mk"][:])
        for w in range(WARMUP):
            wps = psS.tile([128, 128], F32, name="wps", tag="sps")
            mm(wps[:], dtri[:, 0:128], dtri[:, 0:128], start=True, stop=True)

        # DMA order: what attention chunk 0 needs first (wq/wk, x^T chunk 0,
        # wv, masks), then the rest of x^T; W_O last. Intro loads are split
        # across both HWDGE dispatch engines (sync + scalar) since the
        # scalar engine is idle until the first exp. Weights ride single
        # merged descriptors; x^T chunk 0 is two half descriptors, one per
        # queue, so the first qk accumulation chain starts ASAP.
        nc.sync.dma_start(wqm[:], d["wq2"][:])
        for mb in range(4):
            nc.scalar.dma_start(xt_t[mb][:, 0:CH],
                                d["xt"][128 * mb:128 * (mb + 1), 0:CH])
        for mb in range(4, MB):
            nc.sync.dma_start(xt_t[mb][:, 0:CH],
                              d["xt"][128 * mb:128 * (mb + 1), 0:CH])
        nc.scalar.dma_start(wkm[:], d["wk2"][:])
        nc.sync.dma_start(wvm[:], d["wv2"][:])
        if not zero_bias:
            nc.sync.dma_start(wq_b[:], d["wq"][1024:1025, :])
            nc.sync.dma_start(wk_b[:], d["wk"][1024:1025, :])
            nc.sync.dma_start(wv_b[:], d["wv"][1024:1025, :])
            nc.sync.dma_start(xt_ones[:], d["xt"][1024:1025, :])
        nc.scalar.dma_start(
            xsum_t.rearrange("p (m c) -> p m c", c=4),
            d["xs"][0:1024, :].rearrange("(m p) c -> p m c", p=128))
        if not zero_bias:
            nc.scalar.dma_start(xsum_ones[:], d["xs"][1024:1025, :])
        for c in range(1, NCH):
            for mb in range(MB):
                nc.sync.dma_start(
                    xt_t[mb][:, CH * c:CH * (c + 1)],
                    d["xt"][128 * mb:128 * (mb + 1), CH * c:CH * (c + 1)])
        for p in range(NP):
            nc.sync.dma_start(wo_t[p][:], d["wo"][128 * p:128 * (p + 1), :])

        def emit_v(j0):
            # two k-blocks share one [128,512] psum bank: halves the shared
            # psum slot churn. V is packed head-major (256 cols); the
            # per-head ones columns of vt are memset separately.
            ps = psX.tile([128, 2 * 256], F32, name="psv", tag="px")
            for dj in range(2):
                j = j0 + dj
                for mb in range(MB):
                    mm(ps[:, 256 * dj:256 * (dj + 1)],
                       xt_t[mb][:, 128 * j:128 * (j + 1)],
                       wv_t[mb][:], start=(mb == 0),
                       stop=(zero_bias and mb == MB - 1))
                if not zero_bias:
                    mm(ps[:, 256 * dj:256 * (dj + 1)],
                       xt_ones[:, 128 * j:128 * (j + 1)], wv_b[:],
                       start=False, stop=True)
            for dj in range(2):
                j = j0 + dj
                pssl = ps[:, 256 * dj:256 * (dj + 1)]
                nc.vector.tensor_copy(
                    vt[j].rearrange("p (h c) -> p h c", c=65)[:, :, 0:64],
                    pssl.rearrange("p (h c) -> p h c", c=64))
                oc = vt[j].rearrange("p (h c) -> p h c", c=65)[:, :, 64]
                nc.gpsimd.memset(oc, 1.0)

        def emit_qk(p, which, c):
            dst, wt = (qt, wq_t) if which == 0 else (kt, wk_t)
            ps = psX.tile([128, CH], F32, name="psqk", tag="px")
            for mb in range(MB):
                mm(ps[:], wt[mb][:, 128 * p:128 * (p + 1)],
                   xt_t[mb][:, CH * c:CH * (c + 1)],
                   start=(mb == 0), stop=(zero_bias and mb == MB - 1))
            if not zero_bias:
                wb = wq_b if which == 0 else wk_b
                mm(ps[:], wb[:, 128 * p:128 * (p + 1)],
                   xt_ones[:, CH * c:CH * (c + 1)], start=False, stop=True)
            nc.vector.tensor_copy(dst[p][:, CH * c:CH * (c + 1)], ps[:])

        zps_by_pair = {}

        def emit_attn_pair(ch, p):
            # scores/mask/exp/AV for all k-blocks of chunk ch, head pair p
            nj = 4 * ch + 4
            if True:
                h0, h1 = 2 * p, 2 * p + 1
                zps = [psZ.tile([65, CH], F32, name=f"zps{half}",
                                tag=f"zps{half}") for half in range(2)]
                zps_by_pair[(ch, p)] = zps
                for j in range(nj):
                    # both heads of the pair share one [128,1024] scores
                    # psum: one mask-mul + one exp per j. For diagonal
                    # blocks only the unmasked column suffix [w0:512) is
                    # computed; the fully-masked prefix of P is exp(0)=1
                    # exactly, so it is memset to 1.0 (GPSIMD) instead.
                    r = j - 4 * ch
                    w0 = 128 * r if r > 0 else 0
                    last = j == nj - 1
                    sps = psS.tile([128, 2 * CH], F32, name="sps", tag="sps")
                    mm(sps[:, w0:CH],
                       kt[p][0:64, 128 * j:128 * (j + 1)],
                       qt[p][0:64, CH * ch + w0:CH * (ch + 1)],
                       start=True, stop=True)
                    mm(sps[:, CH + w0:2 * CH],
                       kt[p][64:128, 128 * j:128 * (j + 1)],
                       qt[p][64:128, CH * ch + w0:CH * (ch + 1)],
                       start=True, stop=True)
                    sps3 = sps.rearrange("p (t c) -> p t c", t=2)
                    pt = pp.tile([128, 2 * CH], BF16, name="pt", tag="pt")
                    pt3 = pt.rearrange("p (t c) -> p t c", t=2)
                    if r >= 0:
                        # triangular mask on the 128-wide diagonal strip of
                        # both heads at once
                        strip = sps3[:, :, w0:w0 + 128]
                        dtri3 = dtri.rearrange("p (t c) -> p t c", t=2)
                        nc.vector.tensor_mul(strip, strip, dtri3)
                    if w0:
                        nc.gpsimd.memset(pt3[:, :, 0:w0], 1.0)
                        nc.scalar.activation(pt3[:, :, w0:CH],
                                             sps3[:, :, w0:CH], EXP,
                                             scale=0.125)
                    else:
                        nc.scalar.activation(pt[:], sps[:], EXP, scale=0.125)
                    mm(zps[0][:], vt[j][:, 65 * h0:65 * h0 + 65],
                       pt[:, 0:CH], start=(j == 0), stop=last)
                    mm(zps[1][:], vt[j][:, 65 * h1:65 * h1 + 65],
                       pt[:, CH:2 * CH], start=(j == 0), stop=last)

        def emit_sfx():
            # project host-side suffix sums of x through W_V: per pair a
            # [128,4] psum with partition hb+d, col = chunk index, holding
            # sum_{k in suffix(ch)} V[k, d] (+ count*b_V via the ones row)
            for p in range(NP):
                ps = psX.tile([128, 4], F32, name="pssfx", tag="px")
                for mb in range(MB):
                    mm(ps[:], wv_t[mb][:, 128 * p:128 * (p + 1)],
                       xsum_t[:, 4 * mb:4 * (mb + 1)],
                       start=(mb == 0), stop=(zero_bias and mb == MB - 1))
                if not zero_bias:
                    mm(ps[:], wv_b[:, 128 * p:128 * (p + 1)],
                       xsum_ones[:], start=False, stop=True)
                for half in range(2):
                    nc.vector.tensor_copy(sfxT[p][half][:],
                                          ps[64 * half:64 * half + 64, :])

        stg_by_pair = {}

        def emit_zstage(ch, p, tail=False):
            # release-critical: one [65,512] PSUM->SBUF copy per half frees
            # the z psum banks; everything else (reciprocal, broadcast,
            # divide) reads the SBUF staging copy later, off this path.
            # At the tail the scalar engine is idle (no more exps), so the
            # copies run there instead of the loaded vector engine.
            zl = zps_by_pair.pop((ch, p))
            sg = [stg.tile([65, CH], F32, name=f"sg{half}", tag=f"sg{half}")
                  for half in range(2)]
            if tail:
                # stage the denominators first (straight from PSUM, + the
                # suffix count) so the reciprocal chain starts immediately
                cnt = float(S - CH * (ch + 1))
                for half in range(2):
                    nc.scalar.activation(
                        rdcf_t[0:1, CH * half:CH * (half + 1)],
                        zl[half][64:65, :],
                        mybir.ActivationFunctionType.Copy, bias=cnt)
                for half in range(2):
                    nc.scalar.activation(sg[half][:], zl[half][:],
                                         mybir.ActivationFunctionType.Copy)
            else:
                for half in range(2):
                    nc.vector.tensor_copy(sg[half][:], zl[half][:])
            stg_by_pair[(ch, p)] = sg

        def emit_zdiv(ch, p, tail=False):
            # deferred division for (chunk, pair): denominator (+ suffix
            # count) -> reciprocal at partition 0 -> bf16 rows 64/96 ->
            # one 33-row broadcast matmul -> fused (z + sfx) * (1/d) into
            # z2u. Fully-masked suffix k-blocks contribute P=1 each: a
            # count to the denominator and sfxT to z.
            sg = stg_by_pair.pop((ch, p))
            cnt = float(S - CH * (ch + 1))
            rsl = rdcb[:, CH * p:CH * (p + 1)]
            ra = rdcf_t if tail else rdcf_a[:, 2 * CH * p:2 * CH * (p + 1)]
            rb = rdcf_b[:, 2 * CH * p:2 * CH * (p + 1)]
            bc = psX.tile([128, CH], F32, name="bc", tag="px")
            for half, row in ((0, 0), (1, 32)):
                hb = 64 * half
                if not tail:
                    # (at the tail the denominators were staged in zstage)
                    nc.vector.tensor_scalar_add(
                        ra[0:1, CH * half:CH * (half + 1)],
                        sg[half][64:65, :], cnt)
                # per-half reciprocal -> bf16 -> broadcast -> divide so
                # half 0's output is ready while half 1 still processes
                nc.vector.reciprocal_approx_fast(
                    rb[0:1, CH * half:CH * (half + 1)],
                    ra[0:1, CH * half:CH * (half + 1)])
                nc.vector.tensor_copy(rsl[row:row + 1, :],
                                      rb[0:1, CH * half:CH * (half + 1)])
                mm(bc[hb:hb + 64, :], e33[row:row + 1, 64 * half:64 * half + 64],
                   rsl[row:row + 1, :], start=True, stop=True)
                dst = z2u[p][hb:hb + 64, CH * ch:CH * (ch + 1)]
                if cnt:
                    nc.vector.scalar_tensor_tensor(
                        dst, sg[half][0:64, :],
                        sfxT[p][half][:, ch:ch + 1], bc[hb:hb + 64, :],
                        mybir.AluOpType.add, mybir.AluOpType.mult)
                else:
                    nc.vector.tensor_mul(dst, sg[half][0:64, :],
                                         bc[hb:hb + 64, :])

        def emit_divE(ch, tail=False):
            # project chunk ch's z to the output and stream to DRAM. Out
            # DMAs ride the sync queue (dispatching them on the scalar
            # queue delays exp, which gates attention); at the tail the
            # scalar engine+queue are idle, so half the casts and DMAs
            # move there to halve the drain.
            for q in range(4 * ch, 4 * ch + 4):
                for mc in range(2):
                    ops = psX.tile([128, CH], F32, name="ops", tag="px")
                    for p in range(NP):
                        mm(ops[:], z2u[p][:, 128 * q:128 * (q + 1)],
                           wo_t[p][:, CH * mc:CH * (mc + 1)],
                           start=(p == 0), stop=(p == 1))
                    osb = op_sb.tile([128, CH], BF16, name="osb", tag="osb")
                    if tail and mc:
                        nc.scalar.activation(
                            osb[:], ops[:], mybir.ActivationFunctionType.Copy)
                    else:
                        nc.vector.tensor_copy(osb[:], ops[:])
                    eng = nc.scalar if (tail and mc) else nc.sync
                    eng.dma_start(
                        d["out"][128 * q:128 * (q + 1), CH * mc:CH * (mc + 1)],
                        osb[:])

        # ---- emission: emission order doubles as scheduler priority.
        # Per chunk: attention pair 0, its release copies, then the
        # previous chunk's deferred division for pair 0 (fills DVE during
        # this chunk's attention); same for pair 1; then next chunk's Q/K
        # (PE filler at the boundary) and the previous chunk's output
        # projection. The division chains and O-proj run a full pair/chunk
        # behind the attention that produced their data, so the z psum
        # handoff between pairs only ever waits on the two staging copies.
        for p in range(NP):
            emit_qk(p, 0, 0)
            emit_qk(p, 1, 0)
        for j in range(0, 4, 2):
            emit_v(j)
        emit_attn_pair(0, 0)
        for j in range(4, KB, 2):
            emit_v(j)
        emit_sfx()
        emit_zstage(0, 0)
        emit_attn_pair(0, 1)
        emit_zstage(0, 1)
        for ch in range(1, NCH):
            for p in range(NP):
                emit_qk(p, 0, ch)
                emit_qk(p, 1, ch)
            emit_attn_pair(ch, 0)
            emit_zstage(ch, 0)
            emit_zdiv(ch - 1, 0)
            emit_attn_pair(ch, 1)
            emit_zstage(ch, 1, tail=(ch == 3))
            emit_zdiv(ch - 1, 1)
            emit_divE(ch - 1)
        emit_zdiv(3, 0)
        emit_zdiv(3, 1, tail=True)
        emit_divE(3, tail=True)


def build_program(zero_bias=False):
    nc = bacc.Bacc("TRN2", target_bir_lowering=False, debug=False,
                   num_devices=N_CORES)
    d = {
        "xt": nc.dram_tensor("xt", [1025, S], BF16, kind="ExternalInput").ap(),
        "wq": nc.dram_tensor("wq", [1025, 256], BF16, kind="ExternalInput").ap(),
        "wk": nc.dram_tensor("wk", [1025, 256], BF16, kind="ExternalInput").ap(),
        "wv": nc.dram_tensor("wv", [1025, 256], BF16, kind="ExternalInput").ap(),
        "wo": nc.dram_tensor("wo", [256, M], BF16, kind="ExternalInput").ap(),
        "wq2": nc.dram_tensor("wq2", [128, 2048], BF16, kind="ExternalInput").ap(),
        "wk2": nc.dram_tensor("wk2", [128, 2048], BF16, kind="ExternalInput").ap(),
        "wv2": nc.dram_tensor("wv2", [128, 2048], BF16, kind="ExternalInput").ap(),
        "mk": nc.dram_tensor("mk", [128, 256], BF16, kind="ExternalInput").ap(),
        "xs": nc.dram_tensor("xs", [1025, 4], BF16, kind="ExternalInput").ap(),
        "out": nc.dram_tensor("out", [S, M], BF16, kind="ExternalOutput").ap(),
    }
    with tile.TileContext(nc) as tc:
        _emit(tc, nc, d, zero_bias)
    nc.compile()
    return nc


_CACHE = {}


def _get_program(zero_bias=False):
    key = ("nc", zero_bias)
    if key not in _CACHE:
        _CACHE[key] = build_program(zero_bias)
    return _CACHE[key]


def _pack_qk(w4, b4):
    # w4 [4,1024,64], b4 [4,64] -> [1025, 256] (m-major, head-major cols)
    r = np.empty((1025, 256), np.float32)
    r[:1024] = w4.transpose(1, 0, 2).reshape(1024, 256)
    r[1024] = b4.reshape(256)
    return r


def _pack_v(w4, b4):
    # [1025, 256] head-major; vt ones columns are memset on device
    r = np.empty((1025, 256), np.float32)
    r[:1024] = w4.transpose(1, 0, 2).reshape(1024, 256)
    r[1024] = b4.reshape(256)
    return r


def prepare_in_maps(normalized_resid_pre, W_Q, b_Q, W_K, b_K, W_V, b_V, W_O,
                    b_O):
    import ml_dtypes
    bf16 = ml_dtypes.bfloat16
    x = np.asarray(normalized_resid_pre, np.float32)
    W_Q = np.asarray(W_Q, np.float32)
    b_Q = np.asarray(b_Q, np.float32)
    W_K = np.asarray(W_K, np.float32)
    b_K = np.asarray(b_K, np.float32)
    W_V = np.asarray(W_V, np.float32)
    b_V = np.asarray(b_V, np.float32)
    W_O = np.asarray(W_O, np.float32)

    tri = np.triu(np.ones((128, 128), np.float32))  # [k,q]: 1 where k <= q
    mk = np.tile(tri, (1, 2))  # both heads of a pair side by side

    xts = []
    xss = []
    for b in range(2):
        xt = np.empty((1025, S), np.float32)
        xt[:1024] = x[b].T
        xt[1024] = 1.0
        xts.append(xt.astype(bf16))
        # suffix sums of x over k >= 512*(c+1) (input preprocessing for the
        # on-device fully-masked-suffix correction), plus suffix counts in
        # the ones row so the bias matmul picks up count*b_V
        xs = np.zeros((1025, 4), np.float32)
        for c in range(3):
            xs[:1024, c] = x[b][512 * (c + 1):].sum(axis=0)
            xs[1024, c] = S - 512 * (c + 1)
        xss.append(xs.astype(bf16))

    def merge(w):
        # [1024, 256] -> [128, 8*256]: m-block-major columns so the whole
        # projection loads as one DMA descriptor
        return np.ascontiguousarray(
            w[:1024].reshape(8, 128, 256).transpose(1, 0, 2).reshape(128, 2048))

    in_maps = []
    for c in range(N_CORES):
        b, g = divmod(c, 4)
        hs = slice(4 * g, 4 * g + 4)
        wq = _pack_qk(W_Q[hs], b_Q[hs]).astype(bf16)
        wk = _pack_qk(W_K[hs], b_K[hs]).astype(bf16)
        wv = _pack_v(W_V[hs], b_V[hs]).astype(bf16)
        in_maps.append({
            "xt": xts[b],
            "wq": wq, "wk": wk, "wv": wv,
            "wq2": merge(wq), "wk2": merge(wk), "wv2": merge(wv),
            "wo": np.ascontiguousarray(W_O[hs].reshape(256, M)).astype(bf16),
            "mk": mk.astype(bf16),
            "xs": xss[b],
        })
    return in_maps


def gather(results, b_O):
    out = np.zeros((2, S, M), np.float32)
    for c in range(N_CORES):
        out[c // 4] += np.asarray(results[c]["out"], dtype=np.float32)
    out += np.asarray(b_O, np.float32)[None, None, :]
    return out


def _run(in_maps, trace=False, zero_bias=False, **kw):
    nc = _get_program(zero_bias)
    return bass_utils.run_bass_kernel_spmd(
        nc, in_maps, core_ids=list(range(N_CORES)), trace=trace, **kw)


def all_zero_bias(b_Q, b_K, b_V):
    return (not np.any(np.asarray(b_Q)) and not np.any(np.asarray(b_K))
            and not np.any(np.asarray(b_V)))


def kernel(normalized_resid_pre, W_Q, b_Q, W_K, b_K, W_V, b_V, W_O, b_O):
    in_maps = prepare_in_maps(normalized_resid_pre, W_Q, b_Q, W_K, b_K, W_V,
                              b_V, W_O, b_O)
    res = _run(in_maps, zero_bias=all_zero_bias(b_Q, b_K, b_V))
    return gather(res.results, b_O)
